# revision 49
# baseline (speedup 1.0000x reference)
"""Bass/Trainium2 kernel for GQA attention block (nn_FP8Attention).

Full-input contract: kernel(**inputs) takes the complete unsharded inputs and
returns the full [B, S, HIDDEN] output. Internally shards across 8 NeuronCores
as (batch, kv-head-group) pairs: each core handles 1 batch, 1 KV head and its
4 Q heads, computes attention for all 2048 tokens of its batch, then computes
the partial output projection through its heads' rows of wo per 512-token
window and ReduceScatters (sum) the partials within each batch's 4-core group,
leaving each core with the final output for 4x128 of its batch's tokens.

vs the original AllToAll design: x is shipped pre-transposed/pre-cast (no
on-device transposes or f32->bf16 casts), wo is sharded by head rows
(2MB/core instead of full 8MB), and the 8-way AllToAll + staging + full
o-proj tail is replaced by per-window partial o-proj + 4-way ReduceScatter
overlapped with later windows' compute. On-core scheduling: attention scores
run two k-blocks ahead of the PV accumulates (PE never waits on the ACT
exp), causal masking is a post-exp 0/1 multiply on the otherwise-idle gpsimd
queue, the softmax denominator is accumulated elementwise on DVE and
partition-summed by a single ones-matmul per head (instead of re-streaming
every exp block through the PE), each head's normalization chain is emitted
inside the next head's score stream, V is transposed by XBAR DMA from the SP
queue, and PSUM eviction copies ride the ACT engine. Modeled single-core
time: 261us vs 438us for the original (PE-bound, ~78% busy).
"""

import math
import sys
from collections import deque

for _p in ("/opt/trn_rl_repo",):
    if _p not in sys.path:
        sys.path.insert(0, _p)

import numpy as np
import ml_dtypes

import concourse.bass as bass
import concourse.mybir as mybir
import concourse.tile as tile
from concourse import bacc
from concourse.bass_utils import run_bass_kernel_spmd

BF16 = ml_dtypes.bfloat16

B, S, H = 2, 2048, 2048
NH, NKV, HD = 16, 4, 128
P = 128
THETA = 10000.0
NCORES = 8
N_RS = int(__import__("os").environ.get("KERNEL_NRS", "4"))
# timing diagnostic ONLY: replaces collectives with local DMA (wrong output
# for 3/4 of rows) to isolate collective cost from launch/compute cost
NO_CC = __import__("os").environ.get("KERNEL_NOCC", "") == "1"
# disable the w3 remote_dma exchange (fall back to a whole-window collective)
NO_EXCH = __import__("os").environ.get("KERNEL_NOEXCH", "") == "1"

SW = S // 4          # tokens owned per core after ReduceScatter (512)
ISQ = 1.0 / math.sqrt(HD)
HIDC = H // P        # 16 hidden chunks
QHEADS = 4           # q heads per core


def _emit(tc, aps):
    nc = tc.nc
    f32 = mybir.dt.float32
    bf16 = mybir.dt.bfloat16
    Exp = mybir.ActivationFunctionType.Exp

    xT = aps["xT"]
    wqkvT = aps["wqkvT"]
    woTh = aps["woTh"]
    cos_t = aps["cos_t"]
    sin_t = aps["sin_t"]
    rotT = aps["rotT"]
    tri01 = aps["tri01"]
    ones_t = aps["ones_t"]
    padb = aps["padb"]
    y = aps["y"]

    xT_v = xT.rearrange("(hc p) t -> hc p t", p=P)
    wqkvT_v = wqkvT.rearrange("(hc p) o -> hc p o", p=P)
    woTh_v = woTh.rearrange("(h p) o -> h p o", p=P)

    with tc.tile_pool(name="consts", bufs=1) as cp:
        rot_sb = cp.tile([P, P], bf16)
        nc.sync.dma_start(rot_sb, rotT)
        tri01_sb = cp.tile([P, P], bf16)
        nc.sync.dma_start(tri01_sb, tri01)
        ones_sb = cp.tile([P, P], bf16)
        nc.sync.dma_start(ones_sb, ones_t)
        padb_sb = cp.tile([P, HIDC], f32)
        nc.sync.dma_start(padb_sb, padb)

        # weights resident in SBUF; x streamed per 512-token window through a
        # 2-deep rotating buffer (full-x residency was 8MB of SBUF that the
        # w3 remote-exchange buffers now need)
        wqkv_sb = cp.tile([P, HIDC, 768], bf16)
        cos_sb = cp.tile([P, S], bf16)
        sin_sb = cp.tile([P, S], bf16)

        woTh_sb = cp.tile([P, QHEADS, H], bf16)

        # per-window activation tiles; nrm is per-(window, head) so the
        # o-projection's first matmuls don't wait on the last head's norm
        qk = [cp.tile([P, 5, 512], bf16, name=f"qk{w}") for w in range(4)]
        vn = [cp.tile([P, 512], bf16, name=f"vn{w}") for w in range(4)]
        nrm = [[cp.tile([P, 512], bf16, name=f"nrm{w}_{h}") for h in range(QHEADS)]
               for w in range(4)]

        with (
            tc.tile_pool(name="psA", bufs=1, space="PSUM") as psA,
            tc.tile_pool(name="rsd", bufs=1, space="DRAM") as rsd,
            tc.tile_pool(name="ph1", bufs=3) as ph1,
            tc.tile_pool(name="att", bufs=4) as att,
            tc.tile_pool(name="xp", bufs=2) as xp,
        ):
            xw_tiles = {}

            def load_x(w):
                xw = xp.tile([P, HIDC, 512], bf16, tag="xw", name=f"xw{w}")
                tw = slice(w * 512, (w + 1) * 512)
                for hc in range(HIDC):
                    nc.sync.dma_start(xw[:, hc, :], xT_v[hc][:, tw])
                xw_tiles[w] = xw

            # interleave per-hc weight + window-0 x loads so the first
            # projection accumulation can chase the DMA stream instead of
            # waiting for all; window-0 cos/sin slices ride along early for
            # the first rope
            xw0 = xp.tile([P, HIDC, 512], bf16, tag="xw", name="xw0")
            for hc in range(HIDC):
                nc.sync.dma_start(wqkv_sb[:, hc, :], wqkvT_v[hc])
                nc.sync.dma_start(xw0[:, hc, :], xT_v[hc][:, 0:512])
                if hc == 7:
                    nc.sync.dma_start(cos_sb[:, 0:512], cos_t[:, 0:512])
                    nc.sync.dma_start(sin_sb[:, 0:512], sin_t[:, 0:512])
            xw_tiles[0] = xw0
            nc.sync.dma_start(cos_sb[:, 512:2048], cos_t[:, 512:2048])
            nc.sync.dma_start(sin_sb[:, 512:2048], sin_t[:, 512:2048])
            load_x(1)
            for h in range(QHEADS):
                nc.sync.dma_start(woTh_sb[:, h, :], woTh_v[h])
            exchange = (N_RS == 4 and not NO_EXCH
                        and not (aps.get("_single_core") or NO_CC))
            if N_RS == 4:
                # w0-2: whole-window collectives (fully hidden behind later
                # windows' compute). w3 -- the tail-exposed exchange -- uses
                # direct peer SBUF writes (remote_dma_broadcast) + local DVE
                # adds instead: the collective stack costs ~10us fixed + data
                # at ~40-60GB/s on one serialized cc stream (~31-50us for the
                # 2MB window), all after the last compute.
                nw_rs = 3 if exchange else 4
                rs_in = [rsd.tile([4 * P, H], bf16, name=f"rs{w}")
                         for w in range(nw_rs)]
                rs_out = [rsd.tile([P, H], bf16, name=f"rso{w}")
                          for w in range(nw_rs)]
            if exchange:
                # exchange buffers: send3 slot d-1 = my partial for the
                # group-relative peer (Delta-tpb d); acc3 = my own sub's
                # partial; recv3 slot d-1 = peer Delta d's partial for my
                # rows. Replica groups {0..3}/{4..7} are XOR-closed and land
                # on 4-aligned physical NC blocks, so Delta addressing is the
                # same on every core (pure SPMD, no routing tables).
                send3 = cp.tile([P, 3, H], bf16, name="send3")
                acc3 = cp.tile([P, H], bf16, name="acc3")
                recv3 = cp.tile([P, 3, H], bf16, name="recv3")
                # DRAM bounce for the slot repack: predicated DMAs require
                # one side in DRAM
                rs3d = [rsd.tile([P, H], bf16, name=f"rs3d{s}")
                        for s in range(4)]
                rsem3 = nc.alloc_semaphore("rsem3")
                lsem3 = nc.alloc_semaphore("lsem3")
                asem3 = nc.alloc_semaphore("asem3")
                psem3 = nc.alloc_semaphore("psem3")
                # alloc does NOT clear; clear before any peer could write
                # (peers' sends are gated behind 3 full collective rounds of
                # this execution, so a start-of-kernel clear cannot race them)
                nc.gpsimd.sem_clear(rsem3)
                nc.gpsimd.sem_clear(lsem3)
                nc.gpsimd.sem_clear(asem3)
                nc.gpsimd.sem_clear(psem3)
                # group position (device rank mod 4) as a sync-engine runtime
                # value: selects which exchange slot each sub's partial takes
                gv3 = nc.sync.partition_id() % 4
            if N_RS != 4:
                # merged layout: receiver block g' = rows [512g', 512g'+512),
                # window w at rows [512g' + 128w, ...+128) -- one collective
                rs_in_all = rsd.tile([4 * SW, H], bf16, name="rs_all")

            if not (aps.get("_single_core") or NO_CC):
                # tiny warm-up collective issued first: carries the one-time
                # all-core barrier + DMA-ring warmup (observed ~48us barrier
                # + ~19us extra on the first real ReduceScatter) during the
                # initial weight/x DMA phase instead of on the critical path
                warm_in = rsd.tile([4 * P, 16], bf16, name="warm_in")
                warm_out = rsd.tile([P, 16], bf16, name="warm_out")
                for g in range(4):
                    nc.sync.dma_start(warm_in[g * P:(g + 1) * P, 0:16],
                                      rotT[:, 0:16])
                nc.gpsimd.collective_compute(
                    "ReduceScatter",
                    mybir.AluOpType.add,
                    replica_groups=[[0, 1, 2, 3], [4, 5, 6, 7]],
                    ins=[warm_in.opt()],
                    outs=[warm_out.opt()],
                )

            def emit_proj(w, ocs):
                tw = slice(w * 512, (w + 1) * 512)
                if 1 <= w < 3:
                    load_x(w + 1)   # prefetch into the rotating x buffer
                xw = xw_tiles[w]
                # ---- QKV projections for this token window
                for oc in ocs:
                    if oc == 5:
                        # V in natural [token, hd] layout directly: swap the
                        # matmul operands (lhsT = x tokens, rhs = wv columns)
                        # so no transpose is needed at all. The XBAR
                        # DMA-transposes used before are serialized against
                        # collectives by the tile framework
                        # (serialize_transpose_collective_names), which made
                        # each window's PV accumulates wait for the previous
                        # window's ReduceScatter -- ~35-50us PE stall each.
                        ps_v = psA.tile([P, 512], f32, tag="proj", bufs=2)
                        for kt in range(4):
                            for hc in range(HIDC):
                                nc.tensor.matmul(
                                    ps_v[:, kt * P:(kt + 1) * P],
                                    lhsT=xw[:, hc, kt * P:(kt + 1) * P],
                                    rhs=wqkv_sb[:, hc, 640:768],
                                    start=(hc == 0),
                                    stop=(hc == HIDC - 1),
                                    skip_group_check=True,
                                )
                        nc.scalar.copy(vn[w], ps_v)
                        continue
                    ps_p = psA.tile([P, 512], f32, tag="proj", bufs=2)
                    for hc in range(HIDC):
                        nc.tensor.matmul(
                            ps_p,
                            lhsT=wqkv_sb[:, hc, oc * P:(oc + 1) * P],
                            rhs=xw[:, hc, :],
                            start=(hc == 0),
                            stop=(hc == HIDC - 1),
                        )
                    # RoPE: out = q*cos + rot(q)*sin, rot via PE matmul
                    raw = ph1.tile([P, 512], bf16, tag="raw")
                    nc.scalar.copy(raw, ps_p)
                    ps_r = psA.tile([P, 512], f32, tag="rot", bufs=1)
                    nc.tensor.matmul(
                        ps_r, lhsT=rot_sb, rhs=raw, start=True, stop=True
                    )
                    t1 = ph1.tile([P, 512], bf16, tag="t1")
                    nc.vector.tensor_mul(t1, ps_p, cos_sb[:, tw])
                    t2 = ph1.tile([P, 512], bf16, tag="t2")
                    nc.vector.tensor_mul(t2, ps_r, sin_sb[:, tw])
                    nc.vector.tensor_add(qk[w][:, oc, :], t1, t2)
            def emit_attn(w):
                # ---- attention column qc == w for all 4 heads
                qc = w
                n_kc = 4 * qc + 4

                def make_head(h):
                    ps_o = psA.tile([P, 512], f32, tag="o", bufs=2, name="ps_o")
                    # ps_d shares the rope bank: rope uses it only during the
                    # projection phase, ps_d only during attention
                    ps_d = psA.tile([P, 512], f32, tag="rot", bufs=1, name="ps_d")
                    # softmax denominator: accumulate exp blocks elementwise
                    # on DVE (d_acc[p, q] collects k = 128*kc + p), then one
                    # ones-matmul in finalize() does the partition sum --
                    # instead of re-streaming every exp block through the PE.
                    d_acc = att.tile([P, 512], f32, tag="d_acc", bufs=2)

                    def emit_score(kc, first):
                        """scores + exp for one 128-token k block; returns pT.

                        Causal masking: the 128x128 diagonal block is zeroed
                        AFTER exp by a 0/1 triangular multiply on gpsimd (an
                        otherwise idle queue), keeping the PE->ACT exp chain
                        free of DVE round-trips.
                        """
                        b0 = max(0, (kc - 4 * qc) * P)
                        N = 512 - b0
                        kw, kt = divmod(kc, 4)
                        ps_s = psA.tile([P, 512], f32, tag="s", bufs=3)
                        nc.tensor.matmul(
                            ps_s[:, :N],
                            lhsT=qk[kw][:, 4, kt * P:(kt + 1) * P],
                            rhs=qk[qc][:, h, b0:512],
                            start=True,
                            stop=True,
                        )
                        pT = att.tile([P, 512], bf16, tag="pT", bufs=6)
                        nc.scalar.activation(
                            pT[:, :N],
                            ps_s[:, :N],
                            Exp,
                            scale=ISQ,
                            bias=padb_sb[:, kc:kc + 1],
                        )
                        if kc >= 4 * qc:
                            nc.gpsimd.tensor_mul(
                                pT[:, 0:P], pT[:, 0:P], tri01_sb
                            )
                        if first:
                            nc.vector.tensor_copy(d_acc, pT)
                        else:
                            nc.vector.tensor_add(
                                d_acc[:, b0:512], d_acc[:, b0:512], pT[:, :N]
                            )
                        return pT, b0, N, kw, kt

                    def emit_accum(pT, b0, N, kw, kt, first, last):
                        nc.tensor.matmul(
                            ps_o[:, b0:512],
                            lhsT=vn[kw][:, kt * P:(kt + 1) * P],
                            rhs=pT[:, :N],
                            start=first,
                            stop=last,
                            skip_group_check=True,
                        )

                    def run(finalize_prev):
                        # software pipeline: scores run two k-blocks ahead of
                        # the o accumulates so PE never waits on the ACT exp;
                        # the previous head's normalization chain is emitted a
                        # couple of iterations in so its DVE work overlaps
                        # this head's matmuls instead of stalling the PE queue.
                        order = list(range(n_kc))
                        LOOKAHEAD = 2
                        pend = deque()
                        done_fin = finalize_prev is None
                        for pos, kc in enumerate(order):
                            pend.append((emit_score(kc, pos == 0), pos))
                            if len(pend) > LOOKAHEAD:
                                args, p0 = pend.popleft()
                                emit_accum(*args, p0 == 0, p0 == n_kc - 1)
                            if not done_fin and pos >= 1:
                                finalize_prev()
                                done_fin = True
                        if not done_fin:
                            finalize_prev()
                        while pend:
                            args, p0 = pend.popleft()
                            emit_accum(*args, p0 == 0, p0 == n_kc - 1)

                    def finalize():
                        # partition-sum of d_acc via one ones-matmul (bf16
                        # copy first: fp32 rhs would run the PE in fp32 mode)
                        d16 = att.tile([P, 512], bf16, tag="d16", bufs=2)
                        nc.vector.tensor_copy(d16, d_acc)
                        nc.tensor.matmul(
                            ps_d, lhsT=ones_sb, rhs=d16, start=True, stop=True
                        )
                        rec = att.tile([P, 512], f32, tag="rec", bufs=2)
                        nc.vector.reciprocal(rec, ps_d)
                        nc.vector.tensor_mul(nrm[w][h], ps_o, rec)

                    return run, finalize

                fin_prev = None
                for h in range(QHEADS):
                    run_head, fin = make_head(h)
                    run_head(fin_prev)
                    fin_prev = fin
                return fin_prev

            def emit_oproj(w, fin_prev):
                # ---- partial o-proj through this core's 4 head rows of wo.
                # ps_y pairs share the two "s" PSUM banks (free after the
                # attention scores above), and each pair interleaves its
                # h=0..2 accumulations before either h=3 so the PE doesn't
                # sit on the DVE latency of the last head's norm.
                # (sub, fs)/(sub, fs+1) pairing: each sub's [128, 2048]
                # partial completes in turn. w<3 DMA it into rs_in[w] for the
                # collective; w3 cond-DMAs it into the Delta-indexed exchange
                # slot (the one real transfer out of 4 predicated ones --
                # which slot sub s belongs to depends on this core's group
                # position, which is runtime data in an SPMD program).
                split = (w == 3 and exchange)
                yw_tiles = {}
                pairs = [((sub, fs), (sub, fs + 1))
                         for sub in range(4) for fs in (0, 2)]
                for pi, (gA, gB) in enumerate(pairs):
                    ps = {}
                    for g in (gA, gB):
                        ps[g] = psA.tile(
                            [P, 512], f32, tag="s", bufs=3, name="ps_y"
                        )
                    for h in range(QHEADS - 1):
                        for g in (gA, gB):
                            sub, fs = g
                            nc.tensor.matmul(
                                ps[g],
                                lhsT=nrm[w][h][:, sub * P:(sub + 1) * P],
                                rhs=woTh_sb[:, h, fs * 512:(fs + 1) * 512],
                                start=(h == 0),
                                stop=False,
                                skip_group_check=True,
                            )
                        if pi == 0 and h == 1 and fin_prev is not None:
                            # head 3's normalization chain lands here, hidden
                            # behind the first o-proj accumulations
                            fin_prev()
                            fin_prev = None
                    for g in (gA, gB):
                        sub, fs = g
                        h = QHEADS - 1
                        nc.tensor.matmul(
                            ps[g],
                            lhsT=nrm[w][h][:, sub * P:(sub + 1) * P],
                            rhs=woTh_sb[:, h, fs * 512:(fs + 1) * 512],
                            start=False,
                            stop=True,
                            skip_group_check=True,
                        )
                        if sub not in yw_tiles:
                            yw_tiles[sub] = ph1.tile(
                                [P, H], bf16, tag="yw", bufs=2, name="yw"
                            )
                        yw = yw_tiles[sub]
                        nc.scalar.copy(yw[:, fs * 512:(fs + 1) * 512], ps[g])
                        if fs == 3:
                            if split:
                                nc.sync.dma_start(rs3d[sub], yw)
                                for d in range(4):
                                    dst = acc3 if d == 0 else send3[:, d - 1, :]
                                    nc.sync.dma_start(
                                        dst, rs3d[sub],
                                        cond=(gv3 == (sub ^ d)),
                                    )
                            elif N_RS == 4:
                                nc.sync.dma_start(
                                    rs_in[w][sub * P:(sub + 1) * P, :], yw
                                )
                            else:
                                r0 = 512 * sub + P * w
                                nc.sync.dma_start(rs_in_all[r0:r0 + P, :], yw)
                            del yw_tiles[sub]
            def emit_rs(w):
                # ---- ReduceScatter within the 4-core batch group: receiver g
                # gets sum of partials for tokens [512w + 128g, 512w + 128g+128).
                # The y <- rs_out copies are NOT issued here: a sync-queue DMA
                # waiting on the collective would block every later sync-queue
                # op (the next window's V transposes), stalling the PE ~45us
                # per window. All y copies are deferred to the end.
                if N_RS == 4:
                    if aps.get("_single_core") or NO_CC:
                        # timeline-sim stand-in for the collective
                        nc.sync.dma_start(rs_out[w], rs_in[w][0:P, :])
                    else:
                        nc.gpsimd.collective_compute(
                            "ReduceScatter",
                            mybir.AluOpType.add,
                            replica_groups=[[0, 1, 2, 3], [4, 5, 6, 7]],
                            ins=[rs_in[w].opt()],
                            outs=[rs_out[w].opt()],
                        )

            for w in range(4):
                emit_proj(w, range(6))
                fin = emit_attn(w)
                emit_oproj(w, fin)
                if N_RS == 4 and w < nw_rs:
                    emit_rs(w)

            if exchange:
                # ---- w3 peer exchange: three relative-addressed SBUF->SBUF
                # remote writes (all 16 SDMA engines per transfer), then the
                # receiver sums its own partial with the three arrivals.
                # Raw protocol block: remote-sem arrival waits are invisible
                # to the tile scheduler's single-core simulation, so this
                # lives in a critical section (opaque to scheduling; deps on
                # send3/acc3 producers attach to the section boundary).
                yt3 = cp.tile([P, H], bf16, name="yt3")
                with tc.tile_critical():
                    for d in (1, 2, 3):
                        nc.gpsimd.remote_dma_broadcast(
                            recv3[:, d - 1, :],
                            send3[:, d - 1, :],
                            remote_sem=rsem3,
                            local_sem=lsem3,
                            rdests=[(0, d)] * 8,
                        ).then_inc(psem3, 1)
                        # descriptor-write completion must be proven before
                        # the ring doorbell fires (HW desc-gen is async)
                        nc.gpsimd.wait_ge(psem3, d)
                        nc.gpsimd.trigger_dma(count=1)
                    nc.vector.wait_ge(rsem3, 48)
                    nc.vector.tensor_add(
                        yt3, acc3, recv3[:, 0, :]
                    ).then_inc(asem3, 1)
                    nc.vector.wait_ge(asem3, 1)
                    nc.vector.tensor_add(
                        yt3, yt3, recv3[:, 1, :]
                    ).then_inc(asem3, 1)
                    nc.vector.wait_ge(asem3, 2)
                    nc.vector.tensor_add(yt3, yt3, recv3[:, 2, :])
                nc.sync.dma_start(y[3 * P:4 * P, :], yt3)

            if N_RS == 4:
                # final y copies for the collective windows, at the tail of
                # the sync queue where their waits block nothing (the
                # collectives finished long ago)
                for w in range(nw_rs):
                    nc.sync.dma_start(y[w * P:(w + 1) * P, :], rs_out[w])

            if N_RS == 1:
                if aps.get("_single_core") or NO_CC:
                    nc.sync.dma_start(y, rs_in_all[0:SW, :])
                else:
                    nc.gpsimd.collective_compute(
                        "ReduceScatter",
                        mybir.AluOpType.add,
                        replica_groups=[[0, 1, 2, 3], [4, 5, 6, 7]],
                        ins=[rs_in_all.opt()],
                        outs=[y.opt()],
                    )


def build_nc(debug=False, single_core=False):
    nc = bacc.Bacc(
        "TRN2",
        target_bir_lowering=False,
        debug=debug,
        enable_asserts=True,
        num_devices=1 if single_core else NCORES,
    )
    f32 = mybir.dt.float32
    bf16 = mybir.dt.bfloat16
    aps = {
        "xT": nc.dram_tensor("xT", [H, S], bf16, kind="ExternalInput").ap(),
        "wqkvT": nc.dram_tensor("wqkvT", [H, 768], bf16, kind="ExternalInput").ap(),
        "woTh": nc.dram_tensor("woTh", [512, H], bf16, kind="ExternalInput").ap(),
        "cos_t": nc.dram_tensor("cos_t", [P, S], bf16, kind="ExternalInput").ap(),
        "sin_t": nc.dram_tensor("sin_t", [P, S], bf16, kind="ExternalInput").ap(),
        "rotT": nc.dram_tensor("rotT", [P, P], bf16, kind="ExternalInput").ap(),
        "tri01": nc.dram_tensor("tri01", [P, P], bf16, kind="ExternalInput").ap(),
        "ones_t": nc.dram_tensor("ones_t", [P, P], bf16, kind="ExternalInput").ap(),
        "padb": nc.dram_tensor("padb", [P, HIDC], f32, kind="ExternalInput").ap(),
        "y": nc.dram_tensor("y", [SW, H], bf16, kind="ExternalOutput").ap(),
    }
    if single_core:
        aps["_single_core"] = True
    with tile.TileContext(nc) as tc:
        _emit(tc, aps)
    nc.compile()
    return nc


def _to_bf16(a):
    """Fast f32 -> bf16 cast (round-to-nearest-even) via bit manipulation."""
    u = np.ascontiguousarray(a, dtype=np.float32).view(np.uint32)
    r = ((u >> 16) & 1) + np.uint32(0x7FFF)
    return ((u + r) >> 16).astype(np.uint16).view(BF16)


_CONSTS = {}


def _const_tables():
    if _CONSTS:
        return _CONSTS
    pos = np.arange(S, dtype=np.float32)
    inv = 1.0 / THETA ** (np.arange(0, HD, 2, dtype=np.float32) / HD)  # [64]
    ang = inv[:, None] * pos[None, :]                 # [64, S]
    _CONSTS["cos_t"] = np.concatenate(
        [np.cos(ang), np.cos(ang)], axis=0).astype(BF16)
    _CONSTS["sin_t"] = np.concatenate(
        [np.sin(ang), np.sin(ang)], axis=0).astype(BF16)
    A = np.zeros((P, P), dtype=np.float32)
    i = np.arange(64)
    A[i, i + 64] = -1.0
    A[i + 64, i] = 1.0
    _CONSTS["rotT"] = np.ascontiguousarray(A.T).astype(BF16)
    # 0/1 keep-mask for the causal diagonal block: keep q >= k
    _CONSTS["tri01"] = np.where(
        np.arange(P)[None, :] >= np.arange(P)[:, None], 1.0, 0.0
    ).astype(BF16)
    _CONSTS["ones_t"] = np.ones((P, P), dtype=BF16)
    return _CONSTS


def host_inputs(hidden_states, attention_mask, wq, wk, wv, wo):
    """Build the per-core input maps (host-side sharding + constant tables)."""
    hs = np.asarray(hidden_states, dtype=np.float32)
    am = np.asarray(attention_mask)
    wq = np.asarray(wq, dtype=np.float32)
    wk = np.asarray(wk, dtype=np.float32)
    wv = np.asarray(wv, dtype=np.float32)
    wo = np.asarray(wo, dtype=np.float32)
    C = _const_tables()

    # per-batch: pre-transposed bf16 activations + pad bias (shared by 4 cores)
    xT_b, padb_b = [], []
    for b in range(B):
        xT_b.append(np.ascontiguousarray(_to_bf16(hs[b]).T))
        padb = np.where(
            am[b].astype(bool), 0.0, -1e30
        ).astype(np.float32).reshape(HIDC, P).T          # [P, HIDC]
        padb_b.append(np.ascontiguousarray(padb))

    # per-group: qkv + wo-rows weight slices (shared by both batches)
    wqkvT_g, woTh_g = [], []
    for g in range(NKV):
        wqT = wq[4 * g * HD:(4 * g + 4) * HD, :].T       # [H, 512]
        wkT = wk[g * HD:(g + 1) * HD, :].T               # [H, 128]
        wvT = wv[g * HD:(g + 1) * HD, :].T               # [H, 128]
        wqkvT_g.append(np.ascontiguousarray(
            np.concatenate([wqT, wkT, wvT], axis=1)).astype(BF16))
        woTh_g.append(
            np.ascontiguousarray(wo[:, 4 * g * HD:(4 * g + 4) * HD].T).astype(BF16))

    in_maps = []
    for core in range(NCORES):
        b, g = divmod(core, 4)
        in_maps.append(
            {
                "xT": xT_b[b],
                "wqkvT": wqkvT_g[g],
                "woTh": woTh_g[g],
                "cos_t": C["cos_t"],
                "sin_t": C["sin_t"],
                "rotT": C["rotT"],
                "tri01": C["tri01"],
                "ones_t": C["ones_t"],
                "padb": padb_b[b],
            }
        )
    return in_maps


def assemble(results):
    """Gather per-core outputs into the full [B, S, H] array.

    Core (b, g) owns tokens {512*w + 128*g + i} for w in 0..3: its y row
    block w holds the ReduceScattered (summed) output for those tokens.
    """
    out = np.empty((B, S, H), dtype=np.float32)
    for core in range(NCORES):
        b, g = divmod(core, 4)
        yc = np.asarray(results[core]["y"], dtype=np.float32)
        for w in range(4):
            r0 = 512 * w + 128 * g
            out[b, r0:r0 + P, :] = yc[w * P:(w + 1) * P, :]
    return out


_NC_CACHE = {}


def kernel(hidden_states, attention_mask, wq, wk, wv, wo, **run_kwargs):
    in_maps = host_inputs(hidden_states, attention_mask, wq, wk, wv, wo)
    if "nc" not in _NC_CACHE:
        _NC_CACHE["nc"] = build_nc(debug=False)
    nc = _NC_CACHE["nc"]
    res = run_bass_kernel_spmd(nc, in_maps, core_ids=list(range(NCORES)), **run_kwargs)
    out = assemble(res.results)
    kernel.last_results = res
    return out



# revision 65
# speedup vs baseline: 1.1333x; 1.1333x over previous
"""Bass/Trainium2 kernel for GQA attention block (nn_FP8Attention).

Full-input contract: kernel(**inputs) takes the complete unsharded inputs and
returns the full [B, S, HIDDEN] output. Internally shards across 8 NeuronCores
as (batch, kv-head-group) pairs: each core handles 1 batch, 1 KV head and its
4 Q heads, computes attention for all 2048 tokens of its batch, then computes
the partial output projection through its heads' rows of wo per 512-token
window and ReduceScatters (sum) the partials within each batch's 4-core group,
leaving each core with the final output for 4x128 of its batch's tokens.

vs the original AllToAll design: x is shipped pre-transposed/pre-cast (no
on-device transposes or f32->bf16 casts), wo is sharded by head rows
(2MB/core instead of full 8MB), and the 8-way AllToAll + staging + full
o-proj tail is replaced by per-window partial o-proj + 4-way ReduceScatter
overlapped with later windows' compute. On-core scheduling: attention scores
run two k-blocks ahead of the PV accumulates (PE never waits on the ACT
exp), causal masking is a post-exp 0/1 multiply on the otherwise-idle gpsimd
queue, the softmax denominator is accumulated elementwise on DVE and
partition-summed by a single ones-matmul per head (instead of re-streaming
every exp block through the PE), each head's normalization chain is emitted
inside the next head's score stream, V is transposed by XBAR DMA from the SP
queue, and PSUM eviction copies ride the ACT engine. Modeled single-core
time: 261us vs 438us for the original (PE-bound, ~78% busy).
"""

import math
import sys
from collections import deque

for _p in ("/opt/trn_rl_repo",):
    if _p not in sys.path:
        sys.path.insert(0, _p)

import numpy as np
import ml_dtypes

import concourse.bass as bass
import concourse.mybir as mybir
import concourse.tile as tile
from concourse import bacc
from concourse.bass_utils import run_bass_kernel_spmd

BF16 = ml_dtypes.bfloat16

B, S, H = 2, 2048, 2048
NH, NKV, HD = 16, 4, 128
P = 128
THETA = 10000.0
NCORES = 8
N_RS = int(__import__("os").environ.get("KERNEL_NRS", "4"))
# timing diagnostic ONLY: replaces collectives with local DMA (wrong output
# for 3/4 of rows) to isolate collective cost from launch/compute cost
NO_CC = __import__("os").environ.get("KERNEL_NOCC", "") == "1"
# opt-in: replace the w3 tail collective with the remote_dma peer exchange
# (correct on HW in the repeated-dest form but slower; the sliced form hung
# the device -- kept for further debugging only)
USE_EXCH = __import__("os").environ.get("KERNEL_EXCH", "") == "1"

SW = S // 4          # tokens owned per core after ReduceScatter (512)
ISQ = 1.0 / math.sqrt(HD)
HIDC = H // P        # 16 hidden chunks
QHEADS = 4           # q heads per core


def _emit(tc, aps):
    nc = tc.nc
    f32 = mybir.dt.float32
    bf16 = mybir.dt.bfloat16
    Exp = mybir.ActivationFunctionType.Exp

    xT = aps["xT"]
    wqkvT = aps["wqkvT"]
    woTh = aps["woTh"]
    cos_t = aps["cos_t"]
    sin_t = aps["sin_t"]
    rotT = aps["rotT"]
    tri01 = aps["tri01"]
    ones_t = aps["ones_t"]
    padb = aps["padb"]
    y = aps["y"]

    xT_v = xT.rearrange("(hc p) t -> hc p t", p=P)
    wqkvT_v = wqkvT.rearrange("(hc p) o -> hc p o", p=P)
    woTh_v = woTh.rearrange("(h p) o -> h p o", p=P)

    with tc.tile_pool(name="consts", bufs=1) as cp:
        rot_sb = cp.tile([P, P], bf16)
        nc.sync.dma_start(rot_sb, rotT)
        tri01_sb = cp.tile([P, P], bf16)
        nc.sync.dma_start(tri01_sb, tri01)
        ones_sb = cp.tile([P, P], bf16)
        nc.sync.dma_start(ones_sb, ones_t)
        padb_sb = cp.tile([P, HIDC], f32)
        nc.sync.dma_start(padb_sb, padb)

        # weights resident in SBUF; x streamed per 512-token window through a
        # 2-deep rotating buffer (full-x residency was 8MB of SBUF that the
        # w3 remote-exchange buffers now need)
        wqkv_sb = cp.tile([P, HIDC, 768], bf16)
        cos_sb = cp.tile([P, S], bf16)
        sin_sb = cp.tile([P, S], bf16)

        woTh_sb = cp.tile([P, QHEADS, H], bf16)

        # per-window activation tiles; nrm is per-(window, head) so the
        # o-projection's first matmuls don't wait on the last head's norm
        qk = [cp.tile([P, 5, 512], bf16, name=f"qk{w}") for w in range(4)]
        vn = [cp.tile([P, 512], bf16, name=f"vn{w}") for w in range(4)]
        nrm = [[cp.tile([P, 512], bf16, name=f"nrm{w}_{h}") for h in range(QHEADS)]
               for w in range(4)]

        with (
            tc.tile_pool(name="psA", bufs=1, space="PSUM") as psA,
            tc.tile_pool(name="rsd", bufs=1, space="DRAM") as rsd,
            tc.tile_pool(name="ph1", bufs=3) as ph1,
            tc.tile_pool(name="att", bufs=4) as att,
            tc.tile_pool(name="xp", bufs=2) as xp,
        ):
            xw_tiles = {}

            def load_x(w):
                xw = xp.tile([P, HIDC, 512], bf16, tag="xw", name=f"xw{w}")
                tw = slice(w * 512, (w + 1) * 512)
                for hc in range(HIDC):
                    nc.sync.dma_start(xw[:, hc, :], xT_v[hc][:, tw])
                xw_tiles[w] = xw

            # interleave per-hc weight + window-0 x loads so the first
            # projection accumulation can chase the DMA stream instead of
            # waiting for all; window-0 cos/sin slices ride along early for
            # the first rope
            xw0 = xp.tile([P, HIDC, 512], bf16, tag="xw", name="xw0")
            for hc in range(HIDC):
                nc.sync.dma_start(wqkv_sb[:, hc, :], wqkvT_v[hc])
                nc.sync.dma_start(xw0[:, hc, :], xT_v[hc][:, 0:512])
                if hc == 7:
                    nc.sync.dma_start(cos_sb[:, 0:512], cos_t[:, 0:512])
                    nc.sync.dma_start(sin_sb[:, 0:512], sin_t[:, 0:512])
            xw_tiles[0] = xw0
            nc.sync.dma_start(cos_sb[:, 512:2048], cos_t[:, 512:2048])
            nc.sync.dma_start(sin_sb[:, 512:2048], sin_t[:, 512:2048])
            load_x(1)
            for h in range(QHEADS):
                nc.sync.dma_start(woTh_sb[:, h, :], woTh_v[h])
            exchange = (N_RS == 4 and USE_EXCH
                        and not (aps.get("_single_core") or NO_CC))
            if N_RS == 4:
                # w0-2: whole-window collectives (fully hidden behind later
                # windows' compute). w3 -- the tail-exposed exchange -- uses
                # direct peer SBUF writes (remote_dma_broadcast) + local DVE
                # adds instead: the collective stack costs ~10us fixed + data
                # at ~40-60GB/s on one serialized cc stream (~31-50us for the
                # 2MB window), all after the last compute.
                nw_rs = 3
                rs_in = [rsd.tile([4 * P, H], bf16, name=f"rs{w}")
                         for w in range(nw_rs)]
                rs_out = [rsd.tile([P, H], bf16, name=f"rso{w}")
                          for w in range(nw_rs)]
                if not exchange:
                    # w3 tail as two 1MB column-chunk collectives; o-proj w3
                    # runs column-major so chunk 0 launches halfway through
                    rs3_in = [rsd.tile([4 * P, 1024], bf16, name=f"rs3i{c}")
                              for c in range(2)]
                    rs3_out = [rsd.tile([P, 1024], bf16, name=f"rs3o{c}")
                               for c in range(2)]
            if exchange:
                # exchange buffers: send3 slot d-1 = my partial for the
                # group-relative peer (Delta-tpb d); acc3 = my own sub's
                # partial; recv3 slot d-1 = peer Delta d's partial for my
                # rows. Replica groups {0..3}/{4..7} are XOR-closed and land
                # on 4-aligned physical NC blocks, so Delta addressing is the
                # same on every core (pure SPMD, no routing tables).
                send3 = cp.tile([P, 3, H], bf16, name="send3")
                acc3 = cp.tile([P, H], bf16, name="acc3")
                recv3 = cp.tile([P, 3, H], bf16, name="recv3")
                # DRAM bounce for the slot repack: predicated DMAs require
                # one side in DRAM
                rs3d = [rsd.tile([P, H], bf16, name=f"rs3d{s}")
                        for s in range(4)]
                rsem3 = nc.alloc_semaphore("rsem3")
                # local (send-complete) sems are locked to one SWDGE queue
                # each -- one per ring
                lsem3q = [nc.alloc_semaphore(f"lsem3q{q}") for q in range(4)]
                asem3 = nc.alloc_semaphore("asem3")
                psem3 = nc.alloc_semaphore("psem3")
                # alloc does NOT clear; clear before any peer could write
                # (peers' sends are gated behind 3 full collective rounds of
                # this execution, so a start-of-kernel clear cannot race them)
                nc.gpsimd.sem_clear(rsem3)
                for q in range(4):
                    nc.gpsimd.sem_clear(lsem3q[q])
                nc.gpsimd.sem_clear(asem3)
                nc.gpsimd.sem_clear(psem3)
                # group position (device rank mod 4) as a sync-engine runtime
                # value: selects which exchange slot each sub's partial takes
                gv3 = nc.sync.partition_id() % 4
            if N_RS != 4:
                # merged layout: receiver block g' = rows [512g', 512g'+512),
                # window w at rows [512g' + 128w, ...+128) -- one collective
                rs_in_all = rsd.tile([4 * SW, H], bf16, name="rs_all")

            if not (aps.get("_single_core") or NO_CC):
                # tiny warm-up collective issued first: carries the one-time
                # all-core barrier + DMA-ring warmup (observed ~48us barrier
                # + ~19us extra on the first real ReduceScatter) during the
                # initial weight/x DMA phase instead of on the critical path
                warm_in = rsd.tile([4 * P, 16], bf16, name="warm_in")
                warm_out = rsd.tile([P, 16], bf16, name="warm_out")
                for g in range(4):
                    nc.sync.dma_start(warm_in[g * P:(g + 1) * P, 0:16],
                                      rotT[:, 0:16])
                nc.gpsimd.collective_compute(
                    "ReduceScatter",
                    mybir.AluOpType.add,
                    replica_groups=[[0, 1, 2, 3], [4, 5, 6, 7]],
                    ins=[warm_in.opt()],
                    outs=[warm_out.opt()],
                )

            def emit_proj(w, ocs):
                tw = slice(w * 512, (w + 1) * 512)
                if 1 <= w < 3:
                    load_x(w + 1)   # prefetch into the rotating x buffer
                xw = xw_tiles[w]
                # ---- QKV projections for this token window
                for oc in ocs:
                    if oc == 5:
                        # V in natural [token, hd] layout directly: swap the
                        # matmul operands (lhsT = x tokens, rhs = wv columns)
                        # so no transpose is needed at all. The XBAR
                        # DMA-transposes used before are serialized against
                        # collectives by the tile framework
                        # (serialize_transpose_collective_names), which made
                        # each window's PV accumulates wait for the previous
                        # window's ReduceScatter -- ~35-50us PE stall each.
                        ps_v = psA.tile([P, 512], f32, tag="proj", bufs=2)
                        for kt in range(4):
                            for hc in range(HIDC):
                                nc.tensor.matmul(
                                    ps_v[:, kt * P:(kt + 1) * P],
                                    lhsT=xw[:, hc, kt * P:(kt + 1) * P],
                                    rhs=wqkv_sb[:, hc, 640:768],
                                    start=(hc == 0),
                                    stop=(hc == HIDC - 1),
                                    skip_group_check=True,
                                )
                        nc.scalar.copy(vn[w], ps_v)
                        continue
                    ps_p = psA.tile([P, 512], f32, tag="proj", bufs=2)
                    for hc in range(HIDC):
                        nc.tensor.matmul(
                            ps_p,
                            lhsT=wqkv_sb[:, hc, oc * P:(oc + 1) * P],
                            rhs=xw[:, hc, :],
                            start=(hc == 0),
                            stop=(hc == HIDC - 1),
                        )
                    # RoPE: out = q*cos + rot(q)*sin, rot via PE matmul
                    raw = ph1.tile([P, 512], bf16, tag="raw")
                    nc.scalar.copy(raw, ps_p)
                    ps_r = psA.tile([P, 512], f32, tag="rot", bufs=1)
                    nc.tensor.matmul(
                        ps_r, lhsT=rot_sb, rhs=raw, start=True, stop=True
                    )
                    t1 = ph1.tile([P, 512], bf16, tag="t1")
                    nc.vector.tensor_mul(t1, ps_p, cos_sb[:, tw])
                    t2 = ph1.tile([P, 512], bf16, tag="t2")
                    nc.vector.tensor_mul(t2, ps_r, sin_sb[:, tw])
                    nc.vector.tensor_add(qk[w][:, oc, :], t1, t2)
            def emit_attn(w):
                # ---- attention column qc == w for all 4 heads
                qc = w
                n_kc = 4 * qc + 4

                def make_head(h):
                    ps_o = psA.tile([P, 512], f32, tag="o", bufs=2, name="ps_o")
                    # ps_d shares the rope bank: rope uses it only during the
                    # projection phase, ps_d only during attention
                    ps_d = psA.tile([P, 512], f32, tag="rot", bufs=1, name="ps_d")
                    # softmax denominator: accumulate exp blocks elementwise
                    # on DVE (d_acc[p, q] collects k = 128*kc + p), then one
                    # ones-matmul in finalize() does the partition sum --
                    # instead of re-streaming every exp block through the PE.
                    d_acc = att.tile([P, 512], f32, tag="d_acc", bufs=2)

                    def emit_score(kc, first):
                        """scores + exp for one 128-token k block; returns pT.

                        Causal masking: the 128x128 diagonal block is zeroed
                        AFTER exp by a 0/1 triangular multiply on gpsimd (an
                        otherwise idle queue), keeping the PE->ACT exp chain
                        free of DVE round-trips.
                        """
                        b0 = max(0, (kc - 4 * qc) * P)
                        N = 512 - b0
                        kw, kt = divmod(kc, 4)
                        ps_s = psA.tile([P, 512], f32, tag="s", bufs=3)
                        nc.tensor.matmul(
                            ps_s[:, :N],
                            lhsT=qk[kw][:, 4, kt * P:(kt + 1) * P],
                            rhs=qk[qc][:, h, b0:512],
                            start=True,
                            stop=True,
                        )
                        pT = att.tile([P, 512], bf16, tag="pT", bufs=6)
                        nc.scalar.activation(
                            pT[:, :N],
                            ps_s[:, :N],
                            Exp,
                            scale=ISQ,
                            bias=padb_sb[:, kc:kc + 1],
                        )
                        if kc >= 4 * qc:
                            nc.gpsimd.tensor_mul(
                                pT[:, 0:P], pT[:, 0:P], tri01_sb
                            )
                        if first:
                            nc.vector.tensor_copy(d_acc, pT)
                        else:
                            nc.vector.tensor_add(
                                d_acc[:, b0:512], d_acc[:, b0:512], pT[:, :N]
                            )
                        return pT, b0, N, kw, kt

                    def emit_accum(pT, b0, N, kw, kt, first, last):
                        nc.tensor.matmul(
                            ps_o[:, b0:512],
                            lhsT=vn[kw][:, kt * P:(kt + 1) * P],
                            rhs=pT[:, :N],
                            start=first,
                            stop=last,
                            skip_group_check=True,
                        )

                    def run(finalize_prev):
                        # software pipeline: scores run two k-blocks ahead of
                        # the o accumulates so PE never waits on the ACT exp;
                        # the previous head's normalization chain is emitted a
                        # couple of iterations in so its DVE work overlaps
                        # this head's matmuls instead of stalling the PE queue.
                        order = list(range(n_kc))
                        LOOKAHEAD = 2
                        pend = deque()
                        done_fin = finalize_prev is None
                        for pos, kc in enumerate(order):
                            pend.append((emit_score(kc, pos == 0), pos))
                            if len(pend) > LOOKAHEAD:
                                args, p0 = pend.popleft()
                                emit_accum(*args, p0 == 0, p0 == n_kc - 1)
                            if not done_fin and pos >= 1:
                                finalize_prev()
                                done_fin = True
                        if not done_fin:
                            finalize_prev()
                        while pend:
                            args, p0 = pend.popleft()
                            emit_accum(*args, p0 == 0, p0 == n_kc - 1)

                    def finalize():
                        # partition-sum of d_acc via one ones-matmul (bf16
                        # copy first: fp32 rhs would run the PE in fp32 mode)
                        d16 = att.tile([P, 512], bf16, tag="d16", bufs=2)
                        nc.vector.tensor_copy(d16, d_acc)
                        nc.tensor.matmul(
                            ps_d, lhsT=ones_sb, rhs=d16, start=True, stop=True
                        )
                        rec = att.tile([P, 512], f32, tag="rec", bufs=2)
                        nc.vector.reciprocal(rec, ps_d)
                        nc.vector.tensor_mul(nrm[w][h], ps_o, rec)

                    return run, finalize

                fin_prev = None
                for h in range(QHEADS):
                    run_head, fin = make_head(h)
                    run_head(fin_prev)
                    fin_prev = fin
                return fin_prev

            def emit_chunk_rs(c):
                # column-chunk ReduceScatter for window 3: input [512, 1024]
                # (receiver-sub-major rows), output [128, 1024]
                if aps.get("_single_core") or NO_CC:
                    nc.sync.dma_start(rs3_out[c], rs3_in[c][0:P, :])
                else:
                    nc.gpsimd.collective_compute(
                        "ReduceScatter",
                        mybir.AluOpType.add,
                        replica_groups=[[0, 1, 2, 3], [4, 5, 6, 7]],
                        ins=[rs3_in[c].opt()],
                        outs=[rs3_out[c].opt()],
                    )

            def emit_oproj(w, fin_prev):
                # ---- partial o-proj through this core's 4 head rows of wo.
                # ps_y pairs share the two "s" PSUM banks (free after the
                # attention scores above), and each pair interleaves its
                # h=0..2 accumulations before either h=3 so the PE doesn't
                # sit on the DVE latency of the last head's norm.
                # (sub, fs)/(sub, fs+1) pairing: each sub's [128, 2048]
                # partial completes in turn. w<3 DMA it into rs_in[w] for the
                # collective; w3 cond-DMAs it into the Delta-indexed exchange
                # slot (the one real transfer out of 4 predicated ones --
                # which slot sub s belongs to depends on this core's group
                # position, which is runtime data in an SPMD program).
                split = (w == 3 and exchange)
                splitcc = (w == 3 and N_RS == 4 and not exchange)
                yw_tiles = {}
                if splitcc:
                    # fs-major: both halves of column chunk c finish before
                    # chunk c+1 starts, so chunk 0's collective launches at
                    # the o-proj midpoint instead of the end
                    pairs = [((0, f), (1, f)) for f in range(4)]
                    pairs += [((2, f), (3, f)) for f in range(4)]
                    pairs = [pairs[i // 2 + 4 * (i % 2)] for i in range(8)]
                else:
                    pairs = [((sub, fs), (sub, fs + 1))
                             for sub in range(4) for fs in (0, 2)]
                for pi, (gA, gB) in enumerate(pairs):
                    ps = {}
                    for g in (gA, gB):
                        ps[g] = psA.tile(
                            [P, 512], f32, tag="s", bufs=3, name="ps_y"
                        )
                    for h in range(QHEADS - 1):
                        for g in (gA, gB):
                            sub, fs = g
                            nc.tensor.matmul(
                                ps[g],
                                lhsT=nrm[w][h][:, sub * P:(sub + 1) * P],
                                rhs=woTh_sb[:, h, fs * 512:(fs + 1) * 512],
                                start=(h == 0),
                                stop=False,
                                skip_group_check=True,
                            )
                        if pi == 0 and h == 1 and fin_prev is not None:
                            # head 3's normalization chain lands here, hidden
                            # behind the first o-proj accumulations
                            fin_prev()
                            fin_prev = None
                    for g in (gA, gB):
                        sub, fs = g
                        h = QHEADS - 1
                        nc.tensor.matmul(
                            ps[g],
                            lhsT=nrm[w][h][:, sub * P:(sub + 1) * P],
                            rhs=woTh_sb[:, h, fs * 512:(fs + 1) * 512],
                            start=False,
                            stop=True,
                            skip_group_check=True,
                        )
                        if splitcc:
                            yc = ph1.tile([P, 512], bf16, tag="yc", bufs=4,
                                          name="yc")
                            nc.scalar.copy(yc, ps[g])
                            c, fo = divmod(fs, 2)
                            nc.sync.dma_start(
                                rs3_in[c][sub * P:(sub + 1) * P,
                                          fo * 512:(fo + 1) * 512], yc
                            )
                            continue
                        if sub not in yw_tiles:
                            yw_tiles[sub] = ph1.tile(
                                [P, H], bf16, tag="yw", bufs=2, name="yw"
                            )
                        yw = yw_tiles[sub]
                        nc.scalar.copy(yw[:, fs * 512:(fs + 1) * 512], ps[g])
                        if fs == 3:
                            if split:
                                nc.sync.dma_start(rs3d[sub], yw)
                                for d in range(4):
                                    dst = acc3 if d == 0 else send3[:, d - 1, :]
                                    nc.sync.dma_start(
                                        dst, rs3d[sub],
                                        cond=(gv3 == (sub ^ d)),
                                    )
                            elif N_RS == 4:
                                nc.sync.dma_start(
                                    rs_in[w][sub * P:(sub + 1) * P, :], yw
                                )
                            else:
                                r0 = 512 * sub + P * w
                                nc.sync.dma_start(rs_in_all[r0:r0 + P, :], yw)
                            del yw_tiles[sub]
                    if splitcc and pi % 2 == 1 and gA[1] % 2 == 1:
                        emit_chunk_rs(gA[1] // 2)
            def emit_rs(w):
                # ---- ReduceScatter within the 4-core batch group: receiver g
                # gets sum of partials for tokens [512w + 128g, 512w + 128g+128).
                # The y <- rs_out copies are NOT issued here: a sync-queue DMA
                # waiting on the collective would block every later sync-queue
                # op (the next window's V transposes), stalling the PE ~45us
                # per window. All y copies are deferred to the end.
                if N_RS == 4:
                    if aps.get("_single_core") or NO_CC:
                        # timeline-sim stand-in for the collective
                        nc.sync.dma_start(rs_out[w], rs_in[w][0:P, :])
                    else:
                        nc.gpsimd.collective_compute(
                            "ReduceScatter",
                            mybir.AluOpType.add,
                            replica_groups=[[0, 1, 2, 3], [4, 5, 6, 7]],
                            ins=[rs_in[w].opt()],
                            outs=[rs_out[w].opt()],
                        )

            def emit_exch_prep():
                # ---- w3 peer exchange, part 1: generate the 12 SWDGE
                # descriptors (3 peers x 4 column slices, one SDMA engine
                # each via distinct len-16 rdests slots) mid-kernel, where
                # the Q7 library load + desc-gen (~15us) hides behind
                # compute. Descriptors encode addresses only; the data is
                # gated by trigger_dma in the tail critical section.
                with tc.tile_critical():
                    for d in (1, 2, 3):
                        for k in range(4):
                            rd = [None] * 16
                            rd[4 * (d - 1) + k] = (0, d)
                            cs = slice(k * 512, (k + 1) * 512)
                            # spread over the 4 SWDGE rings: 12 preps x 17
                            # descs overflow a single 128-desc ring
                            nc.gpsimd.remote_dma_broadcast(
                                recv3[:, d - 1, cs],
                                send3[:, d - 1, cs],
                                remote_sem=rsem3,
                                local_sem=lsem3q[(4 * (d - 1) + k) % 4],
                                rdests=rd,
                                queue_num=(4 * (d - 1) + k) % 4,
                            ).then_inc(psem3, 1)

            for w in range(4):
                if N_RS == 4 and w >= 2:
                    # y copy for window w-2: its collective completed during
                    # window w-1's compute, so this sync-queue DMA waits on
                    # nothing and the tail only carries the last windows
                    nc.sync.dma_start(y[(w - 2) * P:(w - 1) * P, :],
                                      rs_out[w - 2])
                emit_proj(w, range(6))
                fin = emit_attn(w)
                emit_oproj(w, fin)
                if N_RS == 4 and w < nw_rs:
                    emit_rs(w)
                if w == 1 and exchange:
                    emit_exch_prep()

            if exchange:
                # ---- w3 peer exchange, part 2: fire the pre-generated
                # descriptors and sum. Raw protocol block: remote-sem arrival
                # waits are invisible to the tile scheduler's single-core
                # simulation, so this lives in a critical section.
                yt3 = cp.tile([P, H], bf16, name="yt3")
                probe3 = cp.tile([P, 4], bf16, name="probe3")
                with tc.tile_critical():
                    # probe read: makes the section entry wait on every
                    # send3/acc3 producer (the descriptors reference them,
                    # which the dependency tracker cannot see)
                    nc.gpsimd.tensor_copy(probe3[:, 0:3], send3[:, :, 0])
                    nc.gpsimd.tensor_copy(probe3[:, 3:4], acc3[:, 0:1])
                    nc.gpsimd.wait_ge(psem3, 12)
                    for q in range(4):
                        nc.gpsimd.trigger_dma(count=3, queue_num=q)
                    nc.vector.wait_ge(rsem3, 12)
                    nc.vector.tensor_add(
                        yt3, acc3, recv3[:, 0, :]
                    ).then_inc(asem3, 1)
                    nc.vector.wait_ge(asem3, 1)
                    nc.vector.tensor_add(
                        yt3, yt3, recv3[:, 1, :]
                    ).then_inc(asem3, 1)
                    nc.vector.wait_ge(asem3, 2)
                    nc.vector.tensor_add(yt3, yt3, recv3[:, 2, :])
                nc.sync.dma_start(y[3 * P:4 * P, :], yt3)

            if N_RS == 4:
                # remaining y copies at the tail of the sync queue where
                # their waits block nothing
                nc.sync.dma_start(y[2 * P:3 * P, :], rs_out[2])
                if not exchange:
                    for c in range(2):
                        nc.sync.dma_start(
                            y[3 * P:4 * P, c * 1024:(c + 1) * 1024],
                            rs3_out[c],
                        )

            if N_RS == 1:
                if aps.get("_single_core") or NO_CC:
                    nc.sync.dma_start(y, rs_in_all[0:SW, :])
                else:
                    nc.gpsimd.collective_compute(
                        "ReduceScatter",
                        mybir.AluOpType.add,
                        replica_groups=[[0, 1, 2, 3], [4, 5, 6, 7]],
                        ins=[rs_in_all.opt()],
                        outs=[y.opt()],
                    )


def build_nc(debug=False, single_core=False):
    nc = bacc.Bacc(
        "TRN2",
        target_bir_lowering=False,
        debug=debug,
        enable_asserts=True,
        num_devices=1 if single_core else NCORES,
        num_swdge_queues=4,
    )
    f32 = mybir.dt.float32
    bf16 = mybir.dt.bfloat16
    aps = {
        "xT": nc.dram_tensor("xT", [H, S], bf16, kind="ExternalInput").ap(),
        "wqkvT": nc.dram_tensor("wqkvT", [H, 768], bf16, kind="ExternalInput").ap(),
        "woTh": nc.dram_tensor("woTh", [512, H], bf16, kind="ExternalInput").ap(),
        "cos_t": nc.dram_tensor("cos_t", [P, S], bf16, kind="ExternalInput").ap(),
        "sin_t": nc.dram_tensor("sin_t", [P, S], bf16, kind="ExternalInput").ap(),
        "rotT": nc.dram_tensor("rotT", [P, P], bf16, kind="ExternalInput").ap(),
        "tri01": nc.dram_tensor("tri01", [P, P], bf16, kind="ExternalInput").ap(),
        "ones_t": nc.dram_tensor("ones_t", [P, P], bf16, kind="ExternalInput").ap(),
        "padb": nc.dram_tensor("padb", [P, HIDC], f32, kind="ExternalInput").ap(),
        "y": nc.dram_tensor("y", [SW, H], bf16, kind="ExternalOutput").ap(),
    }
    if single_core:
        aps["_single_core"] = True
    with tile.TileContext(nc) as tc:
        _emit(tc, aps)
    nc.compile()
    return nc


def _to_bf16(a):
    """Fast f32 -> bf16 cast (round-to-nearest-even) via bit manipulation."""
    u = np.ascontiguousarray(a, dtype=np.float32).view(np.uint32)
    r = ((u >> 16) & 1) + np.uint32(0x7FFF)
    return ((u + r) >> 16).astype(np.uint16).view(BF16)


_CONSTS = {}


def _const_tables():
    if _CONSTS:
        return _CONSTS
    pos = np.arange(S, dtype=np.float32)
    inv = 1.0 / THETA ** (np.arange(0, HD, 2, dtype=np.float32) / HD)  # [64]
    ang = inv[:, None] * pos[None, :]                 # [64, S]
    _CONSTS["cos_t"] = np.concatenate(
        [np.cos(ang), np.cos(ang)], axis=0).astype(BF16)
    _CONSTS["sin_t"] = np.concatenate(
        [np.sin(ang), np.sin(ang)], axis=0).astype(BF16)
    A = np.zeros((P, P), dtype=np.float32)
    i = np.arange(64)
    A[i, i + 64] = -1.0
    A[i + 64, i] = 1.0
    _CONSTS["rotT"] = np.ascontiguousarray(A.T).astype(BF16)
    # 0/1 keep-mask for the causal diagonal block: keep q >= k
    _CONSTS["tri01"] = np.where(
        np.arange(P)[None, :] >= np.arange(P)[:, None], 1.0, 0.0
    ).astype(BF16)
    _CONSTS["ones_t"] = np.ones((P, P), dtype=BF16)
    return _CONSTS


def host_inputs(hidden_states, attention_mask, wq, wk, wv, wo):
    """Build the per-core input maps (host-side sharding + constant tables)."""
    hs = np.asarray(hidden_states, dtype=np.float32)
    am = np.asarray(attention_mask)
    wq = np.asarray(wq, dtype=np.float32)
    wk = np.asarray(wk, dtype=np.float32)
    wv = np.asarray(wv, dtype=np.float32)
    wo = np.asarray(wo, dtype=np.float32)
    C = _const_tables()

    # per-batch: pre-transposed bf16 activations + pad bias (shared by 4 cores)
    xT_b, padb_b = [], []
    for b in range(B):
        xT_b.append(np.ascontiguousarray(_to_bf16(hs[b]).T))
        padb = np.where(
            am[b].astype(bool), 0.0, -1e30
        ).astype(np.float32).reshape(HIDC, P).T          # [P, HIDC]
        padb_b.append(np.ascontiguousarray(padb))

    # per-group: qkv + wo-rows weight slices (shared by both batches)
    wqkvT_g, woTh_g = [], []
    for g in range(NKV):
        wqT = wq[4 * g * HD:(4 * g + 4) * HD, :].T       # [H, 512]
        wkT = wk[g * HD:(g + 1) * HD, :].T               # [H, 128]
        wvT = wv[g * HD:(g + 1) * HD, :].T               # [H, 128]
        wqkvT_g.append(np.ascontiguousarray(
            np.concatenate([wqT, wkT, wvT], axis=1)).astype(BF16))
        woTh_g.append(
            np.ascontiguousarray(wo[:, 4 * g * HD:(4 * g + 4) * HD].T).astype(BF16))

    in_maps = []
    for core in range(NCORES):
        b, g = divmod(core, 4)
        in_maps.append(
            {
                "xT": xT_b[b],
                "wqkvT": wqkvT_g[g],
                "woTh": woTh_g[g],
                "cos_t": C["cos_t"],
                "sin_t": C["sin_t"],
                "rotT": C["rotT"],
                "tri01": C["tri01"],
                "ones_t": C["ones_t"],
                "padb": padb_b[b],
            }
        )
    return in_maps


def assemble(results):
    """Gather per-core outputs into the full [B, S, H] array.

    Core (b, g) owns tokens {512*w + 128*g + i} for w in 0..3: its y row
    block w holds the ReduceScattered (summed) output for those tokens.
    """
    out = np.empty((B, S, H), dtype=np.float32)
    for core in range(NCORES):
        b, g = divmod(core, 4)
        yc = np.asarray(results[core]["y"], dtype=np.float32)
        for w in range(4):
            r0 = 512 * w + 128 * g
            out[b, r0:r0 + P, :] = yc[w * P:(w + 1) * P, :]
    return out


_NC_CACHE = {}


def kernel(hidden_states, attention_mask, wq, wk, wv, wo, **run_kwargs):
    in_maps = host_inputs(hidden_states, attention_mask, wq, wk, wv, wo)
    if "nc" not in _NC_CACHE:
        _NC_CACHE["nc"] = build_nc(debug=False)
    nc = _NC_CACHE["nc"]
    res = run_bass_kernel_spmd(nc, in_maps, core_ids=list(range(NCORES)), **run_kwargs)
    out = assemble(res.results)
    kernel.last_results = res
    return out



# revision 67
# speedup vs baseline: 1.1706x; 1.0329x over previous
"""Bass/Trainium2 kernel for GQA attention block (nn_FP8Attention).

Full-input contract: kernel(**inputs) takes the complete unsharded inputs and
returns the full [B, S, HIDDEN] output. Internally shards across 8 NeuronCores
as (batch, kv-head-group) pairs: each core handles 1 batch, 1 KV head and its
4 Q heads, computes attention for all 2048 tokens of its batch, then computes
the partial output projection through its heads' rows of wo per 512-token
window and ReduceScatters (sum) the partials within each batch's 4-core group,
leaving each core with the final output for 4x128 of its batch's tokens.

vs the original AllToAll design: x is shipped pre-transposed/pre-cast (no
on-device transposes or f32->bf16 casts), wo is sharded by head rows
(2MB/core instead of full 8MB), and the 8-way AllToAll + staging + full
o-proj tail is replaced by per-window partial o-proj + 4-way ReduceScatter
overlapped with later windows' compute. On-core scheduling: attention scores
run two k-blocks ahead of the PV accumulates (PE never waits on the ACT
exp), causal masking is a post-exp 0/1 multiply on the otherwise-idle gpsimd
queue, the softmax denominator is accumulated elementwise on DVE and
partition-summed by a single ones-matmul per head (instead of re-streaming
every exp block through the PE), each head's normalization chain is emitted
inside the next head's score stream, V is transposed by XBAR DMA from the SP
queue, and PSUM eviction copies ride the ACT engine. Modeled single-core
time: 261us vs 438us for the original (PE-bound, ~78% busy).
"""

import math
import sys
from collections import deque

for _p in ("/opt/trn_rl_repo",):
    if _p not in sys.path:
        sys.path.insert(0, _p)

import numpy as np
import ml_dtypes

import concourse.bass as bass
import concourse.mybir as mybir
import concourse.tile as tile
from concourse import bacc
from concourse.bass_utils import run_bass_kernel_spmd

BF16 = ml_dtypes.bfloat16

B, S, H = 2, 2048, 2048
NH, NKV, HD = 16, 4, 128
P = 128
THETA = 10000.0
NCORES = 8
N_RS = int(__import__("os").environ.get("KERNEL_NRS", "4"))
# timing diagnostic ONLY: replaces collectives with local DMA (wrong output
# for 3/4 of rows) to isolate collective cost from launch/compute cost
NO_CC = __import__("os").environ.get("KERNEL_NOCC", "") == "1"
# opt-in: replace the w3 tail collective with the remote_dma peer exchange
# (correct on HW in the repeated-dest form but slower; the sliced form hung
# the device -- kept for further debugging only)
USE_EXCH = __import__("os").environ.get("KERNEL_EXCH", "") == "1"

SW = S // 4          # tokens owned per core after ReduceScatter (512)
ISQ = 1.0 / math.sqrt(HD)
HIDC = H // P        # 16 hidden chunks
QHEADS = 4           # q heads per core


def _emit(tc, aps):
    nc = tc.nc
    f32 = mybir.dt.float32
    bf16 = mybir.dt.bfloat16
    Exp = mybir.ActivationFunctionType.Exp

    xT = aps["xT"]
    wqkvT = aps["wqkvT"]
    woTh = aps["woTh"]
    cos_t = aps["cos_t"]
    sin_t = aps["sin_t"]
    rotT = aps["rotT"]
    tri01 = aps["tri01"]
    ones_t = aps["ones_t"]
    padb = aps["padb"]
    y = aps["y"]

    xT_v = xT.rearrange("(hc p) t -> hc p t", p=P)
    wqkvT_v = wqkvT.rearrange("(hc p) o -> hc p o", p=P)
    woTh_v = woTh.rearrange("(h p) o -> h p o", p=P)

    with tc.tile_pool(name="consts", bufs=1) as cp:
        rot_sb = cp.tile([P, P], bf16)
        nc.sync.dma_start(rot_sb, rotT)
        tri01_sb = cp.tile([P, P], bf16)
        nc.sync.dma_start(tri01_sb, tri01)
        ones_sb = cp.tile([P, P], bf16)
        nc.sync.dma_start(ones_sb, ones_t)
        padb_sb = cp.tile([P, HIDC], f32)
        nc.sync.dma_start(padb_sb, padb)

        # weights resident in SBUF; x streamed per 512-token window through a
        # 2-deep rotating buffer (full-x residency was 8MB of SBUF that the
        # w3 remote-exchange buffers now need)
        wqkv_sb = cp.tile([P, HIDC, 768], bf16)
        cos_sb = cp.tile([P, S], bf16)
        sin_sb = cp.tile([P, S], bf16)

        woTh_sb = cp.tile([P, QHEADS, H], bf16)

        # per-window activation tiles; nrm is per-(window, head) so the
        # o-projection's first matmuls don't wait on the last head's norm
        qk = [cp.tile([P, 5, 512], bf16, name=f"qk{w}") for w in range(4)]
        vn = [cp.tile([P, 512], bf16, name=f"vn{w}") for w in range(4)]
        nrm = [[cp.tile([P, 512], bf16, name=f"nrm{w}_{h}") for h in range(QHEADS)]
               for w in range(4)]

        with (
            tc.tile_pool(name="psA", bufs=1, space="PSUM") as psA,
            tc.tile_pool(name="rsd", bufs=1, space="DRAM") as rsd,
            tc.tile_pool(name="ph1", bufs=3) as ph1,
            tc.tile_pool(name="att", bufs=4) as att,
            tc.tile_pool(name="xp", bufs=2) as xp,
        ):
            xw_tiles = {}

            def load_x(w):
                xw = xp.tile([P, HIDC, 512], bf16, tag="xw", name=f"xw{w}")
                tw = slice(w * 512, (w + 1) * 512)
                for hc in range(HIDC):
                    nc.sync.dma_start(xw[:, hc, :], xT_v[hc][:, tw])
                xw_tiles[w] = xw

            # interleave per-hc weight + window-0 x loads so the first
            # projection accumulation can chase the DMA stream instead of
            # waiting for all; window-0 cos/sin slices ride along early for
            # the first rope
            xw0 = xp.tile([P, HIDC, 512], bf16, tag="xw", name="xw0")
            for hc in range(HIDC):
                nc.sync.dma_start(wqkv_sb[:, hc, :], wqkvT_v[hc])
                nc.sync.dma_start(xw0[:, hc, :], xT_v[hc][:, 0:512])
                if hc == 7:
                    nc.sync.dma_start(cos_sb[:, 0:512], cos_t[:, 0:512])
                    nc.sync.dma_start(sin_sb[:, 0:512], sin_t[:, 0:512])
            xw_tiles[0] = xw0
            nc.sync.dma_start(cos_sb[:, 512:2048], cos_t[:, 512:2048])
            nc.sync.dma_start(sin_sb[:, 512:2048], sin_t[:, 512:2048])
            load_x(1)
            for h in range(QHEADS):
                nc.sync.dma_start(woTh_sb[:, h, :], woTh_v[h])
            exchange = (N_RS == 4 and USE_EXCH
                        and not (aps.get("_single_core") or NO_CC))
            if N_RS == 4:
                # w0-2: whole-window collectives (fully hidden behind later
                # windows' compute). w3 -- the tail-exposed exchange -- uses
                # direct peer SBUF writes (remote_dma_broadcast) + local DVE
                # adds instead: the collective stack costs ~10us fixed + data
                # at ~40-60GB/s on one serialized cc stream (~31-50us for the
                # 2MB window), all after the last compute.
                nw_rs = 3
                rs_in = [rsd.tile([4 * P, H], bf16, name=f"rs{w}")
                         for w in range(nw_rs)]
                rs_out = [rsd.tile([P, H], bf16, name=f"rso{w}")
                          for w in range(nw_rs)]
                if not exchange:
                    # w3 tail as two 1MB column-chunk collectives; o-proj w3
                    # runs column-major so chunk 0 launches halfway through
                    rs3_in = [rsd.tile([4 * P, 1024], bf16, name=f"rs3i{c}")
                              for c in range(2)]
                    rs3_out = [rsd.tile([P, 1024], bf16, name=f"rs3o{c}")
                               for c in range(2)]
            if exchange:
                # exchange buffers: send3 slot d-1 = my partial for the
                # group-relative peer (Delta-tpb d); acc3 = my own sub's
                # partial; recv3 slot d-1 = peer Delta d's partial for my
                # rows. Replica groups {0..3}/{4..7} are XOR-closed and land
                # on 4-aligned physical NC blocks, so Delta addressing is the
                # same on every core (pure SPMD, no routing tables).
                send3 = cp.tile([P, 3, H], bf16, name="send3")
                acc3 = cp.tile([P, H], bf16, name="acc3")
                recv3 = cp.tile([P, 3, H], bf16, name="recv3")
                # DRAM bounce for the slot repack: predicated DMAs require
                # one side in DRAM
                rs3d = [rsd.tile([P, H], bf16, name=f"rs3d{s}")
                        for s in range(4)]
                rsem3 = nc.alloc_semaphore("rsem3")
                # local (send-complete) sems are locked to one SWDGE queue
                # each -- one per ring
                lsem3q = [nc.alloc_semaphore(f"lsem3q{q}") for q in range(4)]
                asem3 = nc.alloc_semaphore("asem3")
                psem3 = nc.alloc_semaphore("psem3")
                # alloc does NOT clear; clear before any peer could write
                # (peers' sends are gated behind 3 full collective rounds of
                # this execution, so a start-of-kernel clear cannot race them)
                nc.gpsimd.sem_clear(rsem3)
                for q in range(4):
                    nc.gpsimd.sem_clear(lsem3q[q])
                nc.gpsimd.sem_clear(asem3)
                nc.gpsimd.sem_clear(psem3)
                # group position (device rank mod 4) as a sync-engine runtime
                # value: selects which exchange slot each sub's partial takes
                gv3 = nc.sync.partition_id() % 4
            if N_RS != 4:
                # merged layout: receiver block g' = rows [512g', 512g'+512),
                # window w at rows [512g' + 128w, ...+128) -- one collective
                rs_in_all = rsd.tile([4 * SW, H], bf16, name="rs_all")

            if not (aps.get("_single_core") or NO_CC):
                # tiny warm-up collective issued first: carries the one-time
                # all-core barrier + DMA-ring warmup (observed ~48us barrier
                # + ~19us extra on the first real ReduceScatter) during the
                # initial weight/x DMA phase instead of on the critical path
                warm_in = rsd.tile([4 * P, 16], bf16, name="warm_in")
                warm_out = rsd.tile([P, 16], bf16, name="warm_out")
                for g in range(4):
                    nc.sync.dma_start(warm_in[g * P:(g + 1) * P, 0:16],
                                      rotT[:, 0:16])
                nc.gpsimd.collective_compute(
                    "ReduceScatter",
                    mybir.AluOpType.add,
                    replica_groups=[[0, 1, 2, 3], [4, 5, 6, 7]],
                    ins=[warm_in.opt()],
                    outs=[warm_out.opt()],
                )

            def emit_proj(w, ocs):
                tw = slice(w * 512, (w + 1) * 512)
                if 1 <= w < 3:
                    load_x(w + 1)   # prefetch into the rotating x buffer
                xw = xw_tiles[w]
                # ---- QKV projections for this token window
                for oc in ocs:
                    if oc == 5:
                        # V in natural [token, hd] layout directly: swap the
                        # matmul operands (lhsT = x tokens, rhs = wv columns)
                        # so no transpose is needed at all. The XBAR
                        # DMA-transposes used before are serialized against
                        # collectives by the tile framework
                        # (serialize_transpose_collective_names), which made
                        # each window's PV accumulates wait for the previous
                        # window's ReduceScatter -- ~35-50us PE stall each.
                        ps_v = psA.tile([P, 512], f32, tag="proj", bufs=2)
                        for kt in range(4):
                            for hc in range(HIDC):
                                nc.tensor.matmul(
                                    ps_v[:, kt * P:(kt + 1) * P],
                                    lhsT=xw[:, hc, kt * P:(kt + 1) * P],
                                    rhs=wqkv_sb[:, hc, 640:768],
                                    start=(hc == 0),
                                    stop=(hc == HIDC - 1),
                                    skip_group_check=True,
                                )
                        nc.scalar.copy(vn[w], ps_v)
                        continue
                    ps_p = psA.tile([P, 512], f32, tag="proj", bufs=2)
                    for hc in range(HIDC):
                        nc.tensor.matmul(
                            ps_p,
                            lhsT=wqkv_sb[:, hc, oc * P:(oc + 1) * P],
                            rhs=xw[:, hc, :],
                            start=(hc == 0),
                            stop=(hc == HIDC - 1),
                        )
                    # RoPE: out = q*cos + rot(q)*sin, rot via PE matmul
                    raw = ph1.tile([P, 512], bf16, tag="raw")
                    nc.scalar.copy(raw, ps_p)
                    ps_r = psA.tile([P, 512], f32, tag="rot", bufs=1)
                    nc.tensor.matmul(
                        ps_r, lhsT=rot_sb, rhs=raw, start=True, stop=True
                    )
                    t1 = ph1.tile([P, 512], bf16, tag="t1")
                    nc.vector.tensor_mul(t1, ps_p, cos_sb[:, tw])
                    t2 = ph1.tile([P, 512], bf16, tag="t2")
                    nc.vector.tensor_mul(t2, ps_r, sin_sb[:, tw])
                    nc.vector.tensor_add(qk[w][:, oc, :], t1, t2)
            def emit_attn(w):
                # ---- attention column qc == w for all 4 heads
                qc = w
                n_kc = 4 * qc + 4

                def make_head(h):
                    ps_o = psA.tile([P, 512], f32, tag="o", bufs=2, name="ps_o")
                    # ps_d shares the rope bank: rope uses it only during the
                    # projection phase, ps_d only during attention
                    ps_d = psA.tile([P, 512], f32, tag="rot", bufs=1, name="ps_d")
                    # softmax denominator: accumulate exp blocks elementwise
                    # on DVE (d_acc[p, q] collects k = 128*kc + p), then one
                    # ones-matmul in finalize() does the partition sum --
                    # instead of re-streaming every exp block through the PE.
                    d_acc = att.tile([P, 512], f32, tag="d_acc", bufs=2)

                    def emit_score(kc, first):
                        """scores + exp for one 128-token k block; returns pT.

                        Causal masking: the 128x128 diagonal block is zeroed
                        AFTER exp by a 0/1 triangular multiply on gpsimd (an
                        otherwise idle queue), keeping the PE->ACT exp chain
                        free of DVE round-trips.
                        """
                        b0 = max(0, (kc - 4 * qc) * P)
                        N = 512 - b0
                        kw, kt = divmod(kc, 4)
                        ps_s = psA.tile([P, 512], f32, tag="s", bufs=3)
                        nc.tensor.matmul(
                            ps_s[:, :N],
                            lhsT=qk[kw][:, 4, kt * P:(kt + 1) * P],
                            rhs=qk[qc][:, h, b0:512],
                            start=True,
                            stop=True,
                        )
                        pT = att.tile([P, 512], bf16, tag="pT", bufs=6)
                        nc.scalar.activation(
                            pT[:, :N],
                            ps_s[:, :N],
                            Exp,
                            scale=ISQ,
                            bias=padb_sb[:, kc:kc + 1],
                        )
                        if kc >= 4 * qc:
                            nc.gpsimd.tensor_mul(
                                pT[:, 0:P], pT[:, 0:P], tri01_sb
                            )
                        if first:
                            nc.vector.tensor_copy(d_acc, pT)
                        else:
                            nc.vector.tensor_add(
                                d_acc[:, b0:512], d_acc[:, b0:512], pT[:, :N]
                            )
                        return pT, b0, N, kw, kt

                    def emit_accum(pT, b0, N, kw, kt, first, last):
                        nc.tensor.matmul(
                            ps_o[:, b0:512],
                            lhsT=vn[kw][:, kt * P:(kt + 1) * P],
                            rhs=pT[:, :N],
                            start=first,
                            stop=last,
                            skip_group_check=True,
                        )

                    def run(finalize_prev):
                        # software pipeline: scores run two k-blocks ahead of
                        # the o accumulates so PE never waits on the ACT exp;
                        # the previous head's normalization chain is emitted a
                        # couple of iterations in so its DVE work overlaps
                        # this head's matmuls instead of stalling the PE queue.
                        order = list(range(n_kc))
                        LOOKAHEAD = 2
                        pend = deque()
                        done_fin = finalize_prev is None
                        for pos, kc in enumerate(order):
                            pend.append((emit_score(kc, pos == 0), pos))
                            if len(pend) > LOOKAHEAD:
                                args, p0 = pend.popleft()
                                emit_accum(*args, p0 == 0, p0 == n_kc - 1)
                            if not done_fin and pos >= 1:
                                finalize_prev()
                                done_fin = True
                        if not done_fin:
                            finalize_prev()
                        while pend:
                            args, p0 = pend.popleft()
                            emit_accum(*args, p0 == 0, p0 == n_kc - 1)

                    def finalize():
                        # partition-sum of d_acc via one ones-matmul (bf16
                        # copy first: fp32 rhs would run the PE in fp32 mode)
                        d16 = att.tile([P, 512], bf16, tag="d16", bufs=2)
                        nc.vector.tensor_copy(d16, d_acc)
                        nc.tensor.matmul(
                            ps_d, lhsT=ones_sb, rhs=d16, start=True, stop=True
                        )
                        rec = att.tile([P, 512], f32, tag="rec", bufs=2)
                        nc.vector.reciprocal(rec, ps_d)
                        nc.vector.tensor_mul(nrm[w][h], ps_o, rec)

                    return run, finalize

                fin_prev = None
                for h in range(QHEADS):
                    run_head, fin = make_head(h)
                    run_head(fin_prev)
                    fin_prev = fin
                return fin_prev

            def emit_chunk_rs(c):
                # column-chunk ReduceScatter for window 3: input [512, 1024]
                # (receiver-sub-major rows), output [128, 1024]
                if aps.get("_single_core") or NO_CC:
                    nc.sync.dma_start(rs3_out[c], rs3_in[c][0:P, :])
                else:
                    nc.gpsimd.collective_compute(
                        "ReduceScatter",
                        mybir.AluOpType.add,
                        replica_groups=[[0, 1, 2, 3], [4, 5, 6, 7]],
                        ins=[rs3_in[c].opt()],
                        outs=[rs3_out[c].opt()],
                    )

            def emit_oproj(w, fin_prev):
                # ---- partial o-proj through this core's 4 head rows of wo.
                # ps_y pairs share the two "s" PSUM banks (free after the
                # attention scores above), and each pair interleaves its
                # h=0..2 accumulations before either h=3 so the PE doesn't
                # sit on the DVE latency of the last head's norm.
                # (sub, fs)/(sub, fs+1) pairing: each sub's [128, 2048]
                # partial completes in turn. w<3 DMA it into rs_in[w] for the
                # collective; w3 cond-DMAs it into the Delta-indexed exchange
                # slot (the one real transfer out of 4 predicated ones --
                # which slot sub s belongs to depends on this core's group
                # position, which is runtime data in an SPMD program).
                split = (w == 3 and exchange)
                splitcc = (w == 3 and N_RS == 4 and not exchange)
                yw_tiles = {}
                if splitcc:
                    # fs-major: both halves of column chunk c finish before
                    # chunk c+1 starts, so chunk 0's collective launches at
                    # the o-proj midpoint instead of the end
                    pairs = [((0, f), (1, f)) for f in range(4)]
                    pairs += [((2, f), (3, f)) for f in range(4)]
                    pairs = [pairs[i // 2 + 4 * (i % 2)] for i in range(8)]
                else:
                    pairs = [((sub, fs), (sub, fs + 1))
                             for sub in range(4) for fs in (0, 2)]
                for pi, (gA, gB) in enumerate(pairs):
                    ps = {}
                    for g in (gA, gB):
                        ps[g] = psA.tile(
                            [P, 512], f32, tag="s", bufs=3, name="ps_y"
                        )
                    for h in range(QHEADS - 1):
                        for g in (gA, gB):
                            sub, fs = g
                            nc.tensor.matmul(
                                ps[g],
                                lhsT=nrm[w][h][:, sub * P:(sub + 1) * P],
                                rhs=woTh_sb[:, h, fs * 512:(fs + 1) * 512],
                                start=(h == 0),
                                stop=False,
                                skip_group_check=True,
                            )
                        if pi == 0 and h == 1 and fin_prev is not None:
                            # head 3's normalization chain lands here, hidden
                            # behind the first o-proj accumulations
                            fin_prev()
                            fin_prev = None
                    for g in (gA, gB):
                        sub, fs = g
                        h = QHEADS - 1
                        nc.tensor.matmul(
                            ps[g],
                            lhsT=nrm[w][h][:, sub * P:(sub + 1) * P],
                            rhs=woTh_sb[:, h, fs * 512:(fs + 1) * 512],
                            start=False,
                            stop=True,
                            skip_group_check=True,
                        )
                        if splitcc:
                            yc = ph1.tile([P, 512], bf16, tag="yc", bufs=4,
                                          name="yc")
                            nc.scalar.copy(yc, ps[g])
                            c, fo = divmod(fs, 2)
                            nc.sync.dma_start(
                                rs3_in[c][sub * P:(sub + 1) * P,
                                          fo * 512:(fo + 1) * 512], yc
                            )
                            continue
                        if sub not in yw_tiles:
                            yw_tiles[sub] = ph1.tile(
                                [P, H], bf16, tag="yw", bufs=2, name="yw"
                            )
                        yw = yw_tiles[sub]
                        nc.scalar.copy(yw[:, fs * 512:(fs + 1) * 512], ps[g])
                        if fs == 3:
                            if split:
                                nc.sync.dma_start(rs3d[sub], yw)
                                for d in range(4):
                                    dst = acc3 if d == 0 else send3[:, d - 1, :]
                                    nc.sync.dma_start(
                                        dst, rs3d[sub],
                                        cond=(gv3 == (sub ^ d)),
                                    )
                            elif N_RS == 4:
                                nc.sync.dma_start(
                                    rs_in[w][sub * P:(sub + 1) * P, :], yw
                                )
                            else:
                                r0 = 512 * sub + P * w
                                nc.sync.dma_start(rs_in_all[r0:r0 + P, :], yw)
                            del yw_tiles[sub]
                    if splitcc and pi % 2 == 1 and gA[1] % 2 == 1:
                        emit_chunk_rs(gA[1] // 2)
            def emit_rs(w):
                # ---- ReduceScatter within the 4-core batch group: receiver g
                # gets sum of partials for tokens [512w + 128g, 512w + 128g+128).
                # The y <- rs_out copies are NOT issued here: a sync-queue DMA
                # waiting on the collective would block every later sync-queue
                # op (the next window's V transposes), stalling the PE ~45us
                # per window. All y copies are deferred to the end.
                if N_RS == 4:
                    if aps.get("_single_core") or NO_CC:
                        # timeline-sim stand-in for the collective
                        nc.sync.dma_start(rs_out[w], rs_in[w][0:P, :])
                    else:
                        nc.gpsimd.collective_compute(
                            "ReduceScatter",
                            mybir.AluOpType.add,
                            replica_groups=[[0, 1, 2, 3], [4, 5, 6, 7]],
                            ins=[rs_in[w].opt()],
                            outs=[rs_out[w].opt()],
                        )

            def emit_exch_prep():
                # ---- w3 peer exchange, part 1: generate the 12 SWDGE
                # descriptors (3 peers x 4 column slices, one SDMA engine
                # each via distinct len-16 rdests slots) mid-kernel, where
                # the Q7 library load + desc-gen (~15us) hides behind
                # compute. Descriptors encode addresses only; the data is
                # gated by trigger_dma in the tail critical section.
                with tc.tile_critical():
                    for d in (1, 2, 3):
                        for k in range(4):
                            rd = [None] * 16
                            rd[4 * (d - 1) + k] = (0, d)
                            cs = slice(k * 512, (k + 1) * 512)
                            # spread over the 4 SWDGE rings: 12 preps x 17
                            # descs overflow a single 128-desc ring
                            nc.gpsimd.remote_dma_broadcast(
                                recv3[:, d - 1, cs],
                                send3[:, d - 1, cs],
                                remote_sem=rsem3,
                                local_sem=lsem3q[(4 * (d - 1) + k) % 4],
                                rdests=rd,
                                queue_num=(4 * (d - 1) + k) % 4,
                            ).then_inc(psem3, 1)

            for w in range(4):
                emit_proj(w, range(6))
                fin = emit_attn(w)
                emit_oproj(w, fin)
                if N_RS == 4 and w < nw_rs:
                    emit_rs(w)
                if w == 1 and exchange:
                    emit_exch_prep()

            if exchange:
                # ---- w3 peer exchange, part 2: fire the pre-generated
                # descriptors and sum. Raw protocol block: remote-sem arrival
                # waits are invisible to the tile scheduler's single-core
                # simulation, so this lives in a critical section.
                yt3 = cp.tile([P, H], bf16, name="yt3")
                probe3 = cp.tile([P, 4], bf16, name="probe3")
                with tc.tile_critical():
                    # probe read: makes the section entry wait on every
                    # send3/acc3 producer (the descriptors reference them,
                    # which the dependency tracker cannot see)
                    nc.gpsimd.tensor_copy(probe3[:, 0:3], send3[:, :, 0])
                    nc.gpsimd.tensor_copy(probe3[:, 3:4], acc3[:, 0:1])
                    nc.gpsimd.wait_ge(psem3, 12)
                    for q in range(4):
                        nc.gpsimd.trigger_dma(count=3, queue_num=q)
                    nc.vector.wait_ge(rsem3, 12)
                    nc.vector.tensor_add(
                        yt3, acc3, recv3[:, 0, :]
                    ).then_inc(asem3, 1)
                    nc.vector.wait_ge(asem3, 1)
                    nc.vector.tensor_add(
                        yt3, yt3, recv3[:, 1, :]
                    ).then_inc(asem3, 1)
                    nc.vector.wait_ge(asem3, 2)
                    nc.vector.tensor_add(yt3, yt3, recv3[:, 2, :])
                nc.sync.dma_start(y[3 * P:4 * P, :], yt3)

            if N_RS == 4:
                # y copies at the tail of the sync queue where their waits
                # block nothing (the collectives finished long ago)
                for w in range(nw_rs):
                    nc.sync.dma_start(y[w * P:(w + 1) * P, :], rs_out[w])
                if not exchange:
                    for c in range(2):
                        nc.sync.dma_start(
                            y[3 * P:4 * P, c * 1024:(c + 1) * 1024],
                            rs3_out[c],
                        )

            if N_RS == 1:
                if aps.get("_single_core") or NO_CC:
                    nc.sync.dma_start(y, rs_in_all[0:SW, :])
                else:
                    nc.gpsimd.collective_compute(
                        "ReduceScatter",
                        mybir.AluOpType.add,
                        replica_groups=[[0, 1, 2, 3], [4, 5, 6, 7]],
                        ins=[rs_in_all.opt()],
                        outs=[y.opt()],
                    )


def build_nc(debug=False, single_core=False):
    nc = bacc.Bacc(
        "TRN2",
        target_bir_lowering=False,
        debug=debug,
        enable_asserts=True,
        num_devices=1 if single_core else NCORES,
        num_swdge_queues=4,
    )
    f32 = mybir.dt.float32
    bf16 = mybir.dt.bfloat16
    aps = {
        "xT": nc.dram_tensor("xT", [H, S], bf16, kind="ExternalInput").ap(),
        "wqkvT": nc.dram_tensor("wqkvT", [H, 768], bf16, kind="ExternalInput").ap(),
        "woTh": nc.dram_tensor("woTh", [512, H], bf16, kind="ExternalInput").ap(),
        "cos_t": nc.dram_tensor("cos_t", [P, S], bf16, kind="ExternalInput").ap(),
        "sin_t": nc.dram_tensor("sin_t", [P, S], bf16, kind="ExternalInput").ap(),
        "rotT": nc.dram_tensor("rotT", [P, P], bf16, kind="ExternalInput").ap(),
        "tri01": nc.dram_tensor("tri01", [P, P], bf16, kind="ExternalInput").ap(),
        "ones_t": nc.dram_tensor("ones_t", [P, P], bf16, kind="ExternalInput").ap(),
        "padb": nc.dram_tensor("padb", [P, HIDC], f32, kind="ExternalInput").ap(),
        "y": nc.dram_tensor("y", [SW, H], bf16, kind="ExternalOutput").ap(),
    }
    if single_core:
        aps["_single_core"] = True
    with tile.TileContext(nc) as tc:
        _emit(tc, aps)
    nc.compile()
    return nc


def _to_bf16(a):
    """Fast f32 -> bf16 cast (round-to-nearest-even) via bit manipulation."""
    u = np.ascontiguousarray(a, dtype=np.float32).view(np.uint32)
    r = ((u >> 16) & 1) + np.uint32(0x7FFF)
    return ((u + r) >> 16).astype(np.uint16).view(BF16)


_CONSTS = {}


def _const_tables():
    if _CONSTS:
        return _CONSTS
    pos = np.arange(S, dtype=np.float32)
    inv = 1.0 / THETA ** (np.arange(0, HD, 2, dtype=np.float32) / HD)  # [64]
    ang = inv[:, None] * pos[None, :]                 # [64, S]
    _CONSTS["cos_t"] = np.concatenate(
        [np.cos(ang), np.cos(ang)], axis=0).astype(BF16)
    _CONSTS["sin_t"] = np.concatenate(
        [np.sin(ang), np.sin(ang)], axis=0).astype(BF16)
    A = np.zeros((P, P), dtype=np.float32)
    i = np.arange(64)
    A[i, i + 64] = -1.0
    A[i + 64, i] = 1.0
    _CONSTS["rotT"] = np.ascontiguousarray(A.T).astype(BF16)
    # 0/1 keep-mask for the causal diagonal block: keep q >= k
    _CONSTS["tri01"] = np.where(
        np.arange(P)[None, :] >= np.arange(P)[:, None], 1.0, 0.0
    ).astype(BF16)
    _CONSTS["ones_t"] = np.ones((P, P), dtype=BF16)
    return _CONSTS


def host_inputs(hidden_states, attention_mask, wq, wk, wv, wo):
    """Build the per-core input maps (host-side sharding + constant tables)."""
    hs = np.asarray(hidden_states, dtype=np.float32)
    am = np.asarray(attention_mask)
    wq = np.asarray(wq, dtype=np.float32)
    wk = np.asarray(wk, dtype=np.float32)
    wv = np.asarray(wv, dtype=np.float32)
    wo = np.asarray(wo, dtype=np.float32)
    C = _const_tables()

    # per-batch: pre-transposed bf16 activations + pad bias (shared by 4 cores)
    xT_b, padb_b = [], []
    for b in range(B):
        xT_b.append(np.ascontiguousarray(_to_bf16(hs[b]).T))
        padb = np.where(
            am[b].astype(bool), 0.0, -1e30
        ).astype(np.float32).reshape(HIDC, P).T          # [P, HIDC]
        padb_b.append(np.ascontiguousarray(padb))

    # per-group: qkv + wo-rows weight slices (shared by both batches)
    wqkvT_g, woTh_g = [], []
    for g in range(NKV):
        wqT = wq[4 * g * HD:(4 * g + 4) * HD, :].T       # [H, 512]
        wkT = wk[g * HD:(g + 1) * HD, :].T               # [H, 128]
        wvT = wv[g * HD:(g + 1) * HD, :].T               # [H, 128]
        wqkvT_g.append(np.ascontiguousarray(
            np.concatenate([wqT, wkT, wvT], axis=1)).astype(BF16))
        woTh_g.append(
            np.ascontiguousarray(wo[:, 4 * g * HD:(4 * g + 4) * HD].T).astype(BF16))

    in_maps = []
    for core in range(NCORES):
        b, g = divmod(core, 4)
        in_maps.append(
            {
                "xT": xT_b[b],
                "wqkvT": wqkvT_g[g],
                "woTh": woTh_g[g],
                "cos_t": C["cos_t"],
                "sin_t": C["sin_t"],
                "rotT": C["rotT"],
                "tri01": C["tri01"],
                "ones_t": C["ones_t"],
                "padb": padb_b[b],
            }
        )
    return in_maps


def assemble(results):
    """Gather per-core outputs into the full [B, S, H] array.

    Core (b, g) owns tokens {512*w + 128*g + i} for w in 0..3: its y row
    block w holds the ReduceScattered (summed) output for those tokens.
    """
    out = np.empty((B, S, H), dtype=np.float32)
    for core in range(NCORES):
        b, g = divmod(core, 4)
        yc = np.asarray(results[core]["y"], dtype=np.float32)
        for w in range(4):
            r0 = 512 * w + 128 * g
            out[b, r0:r0 + P, :] = yc[w * P:(w + 1) * P, :]
    return out


_NC_CACHE = {}


def kernel(hidden_states, attention_mask, wq, wk, wv, wo, **run_kwargs):
    in_maps = host_inputs(hidden_states, attention_mask, wq, wk, wv, wo)
    if "nc" not in _NC_CACHE:
        _NC_CACHE["nc"] = build_nc(debug=False)
    nc = _NC_CACHE["nc"]
    res = run_bass_kernel_spmd(nc, in_maps, core_ids=list(range(NCORES)), **run_kwargs)
    out = assemble(res.results)
    kernel.last_results = res
    return out



# revision 70
# speedup vs baseline: 1.2648x; 1.0805x over previous
"""Bass/Trainium2 kernel for GQA attention block (nn_FP8Attention).

Full-input contract: kernel(**inputs) takes the complete unsharded inputs and
returns the full [B, S, HIDDEN] output. Internally shards across 8 NeuronCores
as (batch, kv-head-group) pairs: each core handles 1 batch, 1 KV head and its
4 Q heads, computes attention for all 2048 tokens of its batch, then computes
the partial output projection through its heads' rows of wo per 512-token
window and ReduceScatters (sum) the partials within each batch's 4-core group,
leaving each core with the final output for 4x128 of its batch's tokens.

vs the original AllToAll design: x is shipped pre-transposed/pre-cast (no
on-device transposes or f32->bf16 casts), wo is sharded by head rows
(2MB/core instead of full 8MB), and the 8-way AllToAll + staging + full
o-proj tail is replaced by per-window partial o-proj + 4-way ReduceScatter
overlapped with later windows' compute. On-core scheduling: attention scores
run two k-blocks ahead of the PV accumulates (PE never waits on the ACT
exp), causal masking is a post-exp 0/1 multiply on the otherwise-idle gpsimd
queue, the softmax denominator is accumulated elementwise on DVE and
partition-summed by a single ones-matmul per head (instead of re-streaming
every exp block through the PE), each head's normalization chain is emitted
inside the next head's score stream, V is transposed by XBAR DMA from the SP
queue, and PSUM eviction copies ride the ACT engine. Modeled single-core
time: 261us vs 438us for the original (PE-bound, ~78% busy).
"""

import math
import sys
from collections import deque

for _p in ("/opt/trn_rl_repo",):
    if _p not in sys.path:
        sys.path.insert(0, _p)

import numpy as np
import ml_dtypes

import concourse.bass as bass
import concourse.mybir as mybir
import concourse.tile as tile
from concourse import bacc
from concourse.bass_utils import run_bass_kernel_spmd

BF16 = ml_dtypes.bfloat16

B, S, H = 2, 2048, 2048
NH, NKV, HD = 16, 4, 128
P = 128
THETA = 10000.0
NCORES = 8
N_RS = int(__import__("os").environ.get("KERNEL_NRS", "4"))
# timing diagnostic ONLY: replaces collectives with local DMA (wrong output
# for 3/4 of rows) to isolate collective cost from launch/compute cost
NO_CC = __import__("os").environ.get("KERNEL_NOCC", "") == "1"
# opt-in: replace the w3 tail collective with the remote_dma peer exchange
# (correct on HW in the repeated-dest form but slower; the sliced form hung
# the device -- kept for further debugging only)
USE_EXCH = __import__("os").environ.get("KERNEL_EXCH", "") == "1"

SW = S // 4          # tokens owned per core after ReduceScatter (512)
ISQ = 1.0 / math.sqrt(HD)
HIDC = H // P        # 16 hidden chunks
QHEADS = 4           # q heads per core


def _emit(tc, aps):
    nc = tc.nc
    f32 = mybir.dt.float32
    bf16 = mybir.dt.bfloat16
    Exp = mybir.ActivationFunctionType.Exp

    xT = aps["xT"]
    wqkvT = aps["wqkvT"]
    woTh = aps["woTh"]
    cos_t = aps["cos_t"]
    sin_t = aps["sin_t"]
    rotT = aps["rotT"]
    tri01 = aps["tri01"]
    ones_t = aps["ones_t"]
    padb = aps["padb"]
    y = aps["y"]

    xT_v = xT.rearrange("(hc p) t -> hc p t", p=P)
    wqkvT_v = wqkvT.rearrange("(hc p) o -> hc p o", p=P)
    woTh_v = woTh.rearrange("(h p) o -> h p o", p=P)

    with tc.tile_pool(name="consts", bufs=1) as cp:
        rot_sb = cp.tile([P, P], bf16)
        nc.sync.dma_start(rot_sb, rotT)
        tri01_sb = cp.tile([P, P], bf16)
        nc.sync.dma_start(tri01_sb, tri01)
        ones_sb = cp.tile([P, P], bf16)
        nc.sync.dma_start(ones_sb, ones_t)
        padb_sb = cp.tile([P, HIDC], f32)
        nc.sync.dma_start(padb_sb, padb)

        # weights resident in SBUF; x streamed per 512-token window through a
        # 2-deep rotating buffer (full-x residency was 8MB of SBUF that the
        # w3 remote-exchange buffers now need)
        wqkv_sb = cp.tile([P, HIDC, 768], bf16)
        cos_sb = cp.tile([P, S], bf16)
        sin_sb = cp.tile([P, S], bf16)

        woTh_sb = cp.tile([P, QHEADS, H], bf16)

        # per-window activation tiles; nrm is per-(window, head) so the
        # o-projection's first matmuls don't wait on the last head's norm
        qk = [cp.tile([P, 5, 512], bf16, name=f"qk{w}") for w in range(4)]
        vn = [cp.tile([P, 512], bf16, name=f"vn{w}") for w in range(4)]
        nrm = [[cp.tile([P, 512], bf16, name=f"nrm{w}_{h}") for h in range(QHEADS)]
               for w in range(4)]

        with (
            tc.tile_pool(name="psA", bufs=1, space="PSUM") as psA,
            tc.tile_pool(name="rsd", bufs=1, space="DRAM") as rsd,
            tc.tile_pool(name="ph1", bufs=3) as ph1,
            tc.tile_pool(name="att", bufs=4) as att,
            tc.tile_pool(name="xp", bufs=(2 if USE_EXCH else 4)) as xp,
        ):
            xw_tiles = {}

            def load_x(w):
                xw = xp.tile([P, HIDC, 512], bf16, tag="xw", name=f"xw{w}")
                tw = slice(w * 512, (w + 1) * 512)
                for hc in range(HIDC):
                    nc.sync.dma_start(xw[:, hc, :], xT_v[hc][:, tw])
                xw_tiles[w] = xw

            # interleave per-hc weight + window-0 x loads so the first
            # projection accumulation can chase the DMA stream instead of
            # waiting for all; window-0 cos/sin slices ride along early for
            # the first rope
            xw0 = xp.tile([P, HIDC, 512], bf16, tag="xw", name="xw0")
            for hc in range(HIDC):
                nc.sync.dma_start(wqkv_sb[:, hc, :], wqkvT_v[hc])
                nc.sync.dma_start(xw0[:, hc, :], xT_v[hc][:, 0:512])
                if hc == 7:
                    nc.sync.dma_start(cos_sb[:, 0:512], cos_t[:, 0:512])
                    nc.sync.dma_start(sin_sb[:, 0:512], sin_t[:, 0:512])
            xw_tiles[0] = xw0
            nc.sync.dma_start(cos_sb[:, 512:2048], cos_t[:, 512:2048])
            nc.sync.dma_start(sin_sb[:, 512:2048], sin_t[:, 512:2048])
            load_x(1)
            if not USE_EXCH:
                # x fully resident (8MB; the exchange buffers need the SBUF
                # in exchange mode, where x streams through 2 slots instead)
                load_x(2)
                load_x(3)
            for h in range(QHEADS):
                nc.sync.dma_start(woTh_sb[:, h, :], woTh_v[h])
            exchange = (N_RS == 4 and USE_EXCH
                        and not (aps.get("_single_core") or NO_CC))
            if N_RS == 4:
                # w0-2: whole-window collectives (fully hidden behind later
                # windows' compute). w3 -- the tail-exposed exchange -- uses
                # direct peer SBUF writes (remote_dma_broadcast) + local DVE
                # adds instead: the collective stack costs ~10us fixed + data
                # at ~40-60GB/s on one serialized cc stream (~31-50us for the
                # 2MB window), all after the last compute.
                nw_rs = 3
                rs_in = [rsd.tile([4 * P, H], bf16, name=f"rs{w}")
                         for w in range(nw_rs)]
                rs_out = [rsd.tile([P, H], bf16, name=f"rso{w}")
                          for w in range(nw_rs)]
                if not exchange:
                    # w3 tail as two 1MB column-chunk collectives; o-proj w3
                    # runs column-major so chunk 0 launches halfway through
                    rs3_in = [rsd.tile([4 * P, 1024], bf16, name=f"rs3i{c}")
                              for c in range(2)]
                    rs3_out = [rsd.tile([P, 1024], bf16, name=f"rs3o{c}")
                               for c in range(2)]
            if exchange:
                # exchange buffers: send3 slot d-1 = my partial for the
                # group-relative peer (Delta-tpb d); acc3 = my own sub's
                # partial; recv3 slot d-1 = peer Delta d's partial for my
                # rows. Replica groups {0..3}/{4..7} are XOR-closed and land
                # on 4-aligned physical NC blocks, so Delta addressing is the
                # same on every core (pure SPMD, no routing tables).
                send3 = cp.tile([P, 3, H], bf16, name="send3")
                acc3 = cp.tile([P, H], bf16, name="acc3")
                recv3 = cp.tile([P, 3, H], bf16, name="recv3")
                # DRAM bounce for the slot repack: predicated DMAs require
                # one side in DRAM
                rs3d = [rsd.tile([P, H], bf16, name=f"rs3d{s}")
                        for s in range(4)]
                rsem3 = nc.alloc_semaphore("rsem3")
                # local (send-complete) sems are locked to one SWDGE queue
                # each -- one per ring
                lsem3q = [nc.alloc_semaphore(f"lsem3q{q}") for q in range(4)]
                asem3 = nc.alloc_semaphore("asem3")
                psem3 = nc.alloc_semaphore("psem3")
                # alloc does NOT clear; clear before any peer could write
                # (peers' sends are gated behind 3 full collective rounds of
                # this execution, so a start-of-kernel clear cannot race them)
                nc.gpsimd.sem_clear(rsem3)
                for q in range(4):
                    nc.gpsimd.sem_clear(lsem3q[q])
                nc.gpsimd.sem_clear(asem3)
                nc.gpsimd.sem_clear(psem3)
                # group position (device rank mod 4) as a sync-engine runtime
                # value: selects which exchange slot each sub's partial takes
                gv3 = nc.sync.partition_id() % 4
            if N_RS != 4:
                # merged layout: receiver block g' = rows [512g', 512g'+512),
                # window w at rows [512g' + 128w, ...+128) -- one collective
                rs_in_all = rsd.tile([4 * SW, H], bf16, name="rs_all")

            if not (aps.get("_single_core") or NO_CC):
                # tiny warm-up collective issued first: carries the one-time
                # all-core barrier + DMA-ring warmup (observed ~48us barrier
                # + ~19us extra on the first real ReduceScatter) during the
                # initial weight/x DMA phase instead of on the critical path
                warm_in = rsd.tile([4 * P, 16], bf16, name="warm_in")
                warm_out = rsd.tile([P, 16], bf16, name="warm_out")
                for g in range(4):
                    nc.sync.dma_start(warm_in[g * P:(g + 1) * P, 0:16],
                                      rotT[:, 0:16])
                nc.gpsimd.collective_compute(
                    "ReduceScatter",
                    mybir.AluOpType.add,
                    replica_groups=[[0, 1, 2, 3], [4, 5, 6, 7]],
                    ins=[warm_in.opt()],
                    outs=[warm_out.opt()],
                )

            def emit_proj(w, ocs):
                tw = slice(w * 512, (w + 1) * 512)
                if USE_EXCH and 1 <= w < 3:
                    load_x(w + 1)   # prefetch into the rotating x buffer
                xw = xw_tiles[w]
                # ---- QKV projections for this token window
                for oc in ocs:
                    if oc == 5:
                        # V in natural [token, hd] layout directly: swap the
                        # matmul operands (lhsT = x tokens, rhs = wv columns)
                        # so no transpose is needed at all. The XBAR
                        # DMA-transposes used before are serialized against
                        # collectives by the tile framework
                        # (serialize_transpose_collective_names), which made
                        # each window's PV accumulates wait for the previous
                        # window's ReduceScatter -- ~35-50us PE stall each.
                        ps_v = psA.tile([P, 512], f32, tag="proj", bufs=2)
                        for kt in range(4):
                            for hc in range(HIDC):
                                nc.tensor.matmul(
                                    ps_v[:, kt * P:(kt + 1) * P],
                                    lhsT=xw[:, hc, kt * P:(kt + 1) * P],
                                    rhs=wqkv_sb[:, hc, 640:768],
                                    start=(hc == 0),
                                    stop=(hc == HIDC - 1),
                                    skip_group_check=True,
                                )
                        nc.scalar.copy(vn[w], ps_v)
                        continue
                    ps_p = psA.tile([P, 512], f32, tag="proj", bufs=2)
                    for hc in range(HIDC):
                        nc.tensor.matmul(
                            ps_p,
                            lhsT=wqkv_sb[:, hc, oc * P:(oc + 1) * P],
                            rhs=xw[:, hc, :],
                            start=(hc == 0),
                            stop=(hc == HIDC - 1),
                        )
                    # RoPE: out = q*cos + rot(q)*sin, rot via PE matmul
                    raw = ph1.tile([P, 512], bf16, tag="raw")
                    nc.scalar.copy(raw, ps_p)
                    ps_r = psA.tile([P, 512], f32, tag="rot", bufs=1)
                    nc.tensor.matmul(
                        ps_r, lhsT=rot_sb, rhs=raw, start=True, stop=True
                    )
                    t1 = ph1.tile([P, 512], bf16, tag="t1")
                    nc.vector.tensor_mul(t1, ps_p, cos_sb[:, tw])
                    t2 = ph1.tile([P, 512], bf16, tag="t2")
                    nc.vector.tensor_mul(t2, ps_r, sin_sb[:, tw])
                    nc.vector.tensor_add(qk[w][:, oc, :], t1, t2)
            def emit_attn(w):
                # ---- attention column qc == w for all 4 heads
                qc = w
                n_kc = 4 * qc + 4

                def make_head(h):
                    ps_o = psA.tile([P, 512], f32, tag="o", bufs=2, name="ps_o")
                    # ps_d shares the rope bank: rope uses it only during the
                    # projection phase, ps_d only during attention
                    ps_d = psA.tile([P, 512], f32, tag="rot", bufs=1, name="ps_d")
                    # softmax denominator: accumulate exp blocks elementwise
                    # on DVE (d_acc[p, q] collects k = 128*kc + p), then one
                    # ones-matmul in finalize() does the partition sum --
                    # instead of re-streaming every exp block through the PE.
                    d_acc = att.tile([P, 512], f32, tag="d_acc", bufs=2)

                    def emit_score(kc, first):
                        """scores + exp for one 128-token k block; returns pT.

                        Causal masking: the 128x128 diagonal block is zeroed
                        AFTER exp by a 0/1 triangular multiply on gpsimd (an
                        otherwise idle queue), keeping the PE->ACT exp chain
                        free of DVE round-trips.
                        """
                        b0 = max(0, (kc - 4 * qc) * P)
                        N = 512 - b0
                        kw, kt = divmod(kc, 4)
                        ps_s = psA.tile([P, 512], f32, tag="s", bufs=3)
                        nc.tensor.matmul(
                            ps_s[:, :N],
                            lhsT=qk[kw][:, 4, kt * P:(kt + 1) * P],
                            rhs=qk[qc][:, h, b0:512],
                            start=True,
                            stop=True,
                        )
                        pT = att.tile([P, 512], bf16, tag="pT", bufs=6)
                        nc.scalar.activation(
                            pT[:, :N],
                            ps_s[:, :N],
                            Exp,
                            scale=ISQ,
                            bias=padb_sb[:, kc:kc + 1],
                        )
                        if kc >= 4 * qc:
                            nc.gpsimd.tensor_mul(
                                pT[:, 0:P], pT[:, 0:P], tri01_sb
                            )
                        if first:
                            nc.vector.tensor_copy(d_acc, pT)
                        else:
                            nc.vector.tensor_add(
                                d_acc[:, b0:512], d_acc[:, b0:512], pT[:, :N]
                            )
                        return pT, b0, N, kw, kt

                    def emit_accum(pT, b0, N, kw, kt, first, last):
                        nc.tensor.matmul(
                            ps_o[:, b0:512],
                            lhsT=vn[kw][:, kt * P:(kt + 1) * P],
                            rhs=pT[:, :N],
                            start=first,
                            stop=last,
                            skip_group_check=True,
                        )

                    def run(finalize_prev):
                        # software pipeline: scores run two k-blocks ahead of
                        # the o accumulates so PE never waits on the ACT exp;
                        # the previous head's normalization chain is emitted a
                        # couple of iterations in so its DVE work overlaps
                        # this head's matmuls instead of stalling the PE queue.
                        order = list(range(n_kc))
                        LOOKAHEAD = 2
                        pend = deque()
                        done_fin = finalize_prev is None
                        for pos, kc in enumerate(order):
                            pend.append((emit_score(kc, pos == 0), pos))
                            if len(pend) > LOOKAHEAD:
                                args, p0 = pend.popleft()
                                emit_accum(*args, p0 == 0, p0 == n_kc - 1)
                            if not done_fin and pos >= 1:
                                finalize_prev()
                                done_fin = True
                        if not done_fin:
                            finalize_prev()
                        while pend:
                            args, p0 = pend.popleft()
                            emit_accum(*args, p0 == 0, p0 == n_kc - 1)

                    def finalize():
                        # partition-sum of d_acc via one ones-matmul (bf16
                        # copy first: fp32 rhs would run the PE in fp32 mode)
                        d16 = att.tile([P, 512], bf16, tag="d16", bufs=2)
                        nc.vector.tensor_copy(d16, d_acc)
                        nc.tensor.matmul(
                            ps_d, lhsT=ones_sb, rhs=d16, start=True, stop=True
                        )
                        rec = att.tile([P, 512], f32, tag="rec", bufs=2)
                        nc.vector.reciprocal(rec, ps_d)
                        nc.vector.tensor_mul(nrm[w][h], ps_o, rec)

                    return run, finalize

                fin_prev = None
                for h in range(QHEADS):
                    run_head, fin = make_head(h)
                    run_head(fin_prev)
                    fin_prev = fin
                return fin_prev

            def emit_chunk_rs(c):
                # column-chunk ReduceScatter for window 3: input [512, 1024]
                # (receiver-sub-major rows), output [128, 1024]
                if aps.get("_single_core") or NO_CC:
                    nc.sync.dma_start(rs3_out[c], rs3_in[c][0:P, :])
                else:
                    nc.gpsimd.collective_compute(
                        "ReduceScatter",
                        mybir.AluOpType.add,
                        replica_groups=[[0, 1, 2, 3], [4, 5, 6, 7]],
                        ins=[rs3_in[c].opt()],
                        outs=[rs3_out[c].opt()],
                    )

            def emit_oproj(w, fin_prev):
                # ---- partial o-proj through this core's 4 head rows of wo.
                # ps_y pairs share the two "s" PSUM banks (free after the
                # attention scores above), and each pair interleaves its
                # h=0..2 accumulations before either h=3 so the PE doesn't
                # sit on the DVE latency of the last head's norm.
                # (sub, fs)/(sub, fs+1) pairing: each sub's [128, 2048]
                # partial completes in turn. w<3 DMA it into rs_in[w] for the
                # collective; w3 cond-DMAs it into the Delta-indexed exchange
                # slot (the one real transfer out of 4 predicated ones --
                # which slot sub s belongs to depends on this core's group
                # position, which is runtime data in an SPMD program).
                split = (w == 3 and exchange)
                splitcc = (w == 3 and N_RS == 4 and not exchange)
                yw_tiles = {}
                if splitcc:
                    # fs-major: both halves of column chunk c finish before
                    # chunk c+1 starts, so chunk 0's collective launches at
                    # the o-proj midpoint instead of the end
                    pairs = [((0, f), (1, f)) for f in range(4)]
                    pairs += [((2, f), (3, f)) for f in range(4)]
                    pairs = [pairs[i // 2 + 4 * (i % 2)] for i in range(8)]
                else:
                    pairs = [((sub, fs), (sub, fs + 1))
                             for sub in range(4) for fs in (0, 2)]
                for pi, (gA, gB) in enumerate(pairs):
                    ps = {}
                    for g in (gA, gB):
                        ps[g] = psA.tile(
                            [P, 512], f32, tag="s", bufs=3, name="ps_y"
                        )
                    for h in range(QHEADS - 1):
                        for g in (gA, gB):
                            sub, fs = g
                            nc.tensor.matmul(
                                ps[g],
                                lhsT=nrm[w][h][:, sub * P:(sub + 1) * P],
                                rhs=woTh_sb[:, h, fs * 512:(fs + 1) * 512],
                                start=(h == 0),
                                stop=False,
                                skip_group_check=True,
                            )
                        if pi == 0 and h == 1 and fin_prev is not None:
                            # head 3's normalization chain lands here, hidden
                            # behind the first o-proj accumulations
                            fin_prev()
                            fin_prev = None
                    for g in (gA, gB):
                        sub, fs = g
                        h = QHEADS - 1
                        nc.tensor.matmul(
                            ps[g],
                            lhsT=nrm[w][h][:, sub * P:(sub + 1) * P],
                            rhs=woTh_sb[:, h, fs * 512:(fs + 1) * 512],
                            start=False,
                            stop=True,
                            skip_group_check=True,
                        )
                        if splitcc:
                            yc = ph1.tile([P, 512], bf16, tag="yc", bufs=4,
                                          name="yc")
                            nc.scalar.copy(yc, ps[g])
                            c, fo = divmod(fs, 2)
                            nc.sync.dma_start(
                                rs3_in[c][sub * P:(sub + 1) * P,
                                          fo * 512:(fo + 1) * 512], yc
                            )
                            continue
                        if sub not in yw_tiles:
                            yw_tiles[sub] = ph1.tile(
                                [P, H], bf16, tag="yw", bufs=2, name="yw"
                            )
                        yw = yw_tiles[sub]
                        nc.scalar.copy(yw[:, fs * 512:(fs + 1) * 512], ps[g])
                        if fs == 3:
                            if split:
                                nc.sync.dma_start(rs3d[sub], yw)
                                for d in range(4):
                                    dst = acc3 if d == 0 else send3[:, d - 1, :]
                                    nc.sync.dma_start(
                                        dst, rs3d[sub],
                                        cond=(gv3 == (sub ^ d)),
                                    )
                            elif N_RS == 4:
                                nc.sync.dma_start(
                                    rs_in[w][sub * P:(sub + 1) * P, :], yw
                                )
                            else:
                                r0 = 512 * sub + P * w
                                nc.sync.dma_start(rs_in_all[r0:r0 + P, :], yw)
                            del yw_tiles[sub]
                    if splitcc and pi % 2 == 1 and gA[1] % 2 == 1:
                        emit_chunk_rs(gA[1] // 2)
            def emit_rs(w):
                # ---- ReduceScatter within the 4-core batch group: receiver g
                # gets sum of partials for tokens [512w + 128g, 512w + 128g+128).
                # The y <- rs_out copies are NOT issued here: a sync-queue DMA
                # waiting on the collective would block every later sync-queue
                # op (the next window's V transposes), stalling the PE ~45us
                # per window. All y copies are deferred to the end.
                if N_RS == 4:
                    if aps.get("_single_core") or NO_CC:
                        # timeline-sim stand-in for the collective
                        nc.sync.dma_start(rs_out[w], rs_in[w][0:P, :])
                    else:
                        nc.gpsimd.collective_compute(
                            "ReduceScatter",
                            mybir.AluOpType.add,
                            replica_groups=[[0, 1, 2, 3], [4, 5, 6, 7]],
                            ins=[rs_in[w].opt()],
                            outs=[rs_out[w].opt()],
                        )

            def emit_exch_prep():
                # ---- w3 peer exchange, part 1: generate the 12 SWDGE
                # descriptors (3 peers x 4 column slices, one SDMA engine
                # each via distinct len-16 rdests slots) mid-kernel, where
                # the Q7 library load + desc-gen (~15us) hides behind
                # compute. Descriptors encode addresses only; the data is
                # gated by trigger_dma in the tail critical section.
                with tc.tile_critical():
                    for d in (1, 2, 3):
                        for k in range(4):
                            rd = [None] * 16
                            rd[4 * (d - 1) + k] = (0, d)
                            cs = slice(k * 512, (k + 1) * 512)
                            # spread over the 4 SWDGE rings: 12 preps x 17
                            # descs overflow a single 128-desc ring
                            nc.gpsimd.remote_dma_broadcast(
                                recv3[:, d - 1, cs],
                                send3[:, d - 1, cs],
                                remote_sem=rsem3,
                                local_sem=lsem3q[(4 * (d - 1) + k) % 4],
                                rdests=rd,
                                queue_num=(4 * (d - 1) + k) % 4,
                            ).then_inc(psem3, 1)

            for w in range(4):
                emit_proj(w, range(6))
                fin = emit_attn(w)
                emit_oproj(w, fin)
                if N_RS == 4 and w < nw_rs:
                    emit_rs(w)
                if w == 1 and exchange:
                    emit_exch_prep()

            if exchange:
                # ---- w3 peer exchange, part 2: fire the pre-generated
                # descriptors and sum. Raw protocol block: remote-sem arrival
                # waits are invisible to the tile scheduler's single-core
                # simulation, so this lives in a critical section.
                yt3 = cp.tile([P, H], bf16, name="yt3")
                probe3 = cp.tile([P, 4], bf16, name="probe3")
                with tc.tile_critical():
                    # probe read: makes the section entry wait on every
                    # send3/acc3 producer (the descriptors reference them,
                    # which the dependency tracker cannot see)
                    nc.gpsimd.tensor_copy(probe3[:, 0:3], send3[:, :, 0])
                    nc.gpsimd.tensor_copy(probe3[:, 3:4], acc3[:, 0:1])
                    nc.gpsimd.wait_ge(psem3, 12)
                    for q in range(4):
                        nc.gpsimd.trigger_dma(count=3, queue_num=q)
                    nc.vector.wait_ge(rsem3, 12)
                    nc.vector.tensor_add(
                        yt3, acc3, recv3[:, 0, :]
                    ).then_inc(asem3, 1)
                    nc.vector.wait_ge(asem3, 1)
                    nc.vector.tensor_add(
                        yt3, yt3, recv3[:, 1, :]
                    ).then_inc(asem3, 1)
                    nc.vector.wait_ge(asem3, 2)
                    nc.vector.tensor_add(yt3, yt3, recv3[:, 2, :])
                nc.sync.dma_start(y[3 * P:4 * P, :], yt3)

            if N_RS == 4:
                # y copies at the tail of the sync queue where their waits
                # block nothing (the collectives finished long ago)
                for w in range(nw_rs):
                    nc.sync.dma_start(y[w * P:(w + 1) * P, :], rs_out[w])
                if not exchange:
                    for c in range(2):
                        nc.sync.dma_start(
                            y[3 * P:4 * P, c * 1024:(c + 1) * 1024],
                            rs3_out[c],
                        )

            if N_RS == 1:
                if aps.get("_single_core") or NO_CC:
                    nc.sync.dma_start(y, rs_in_all[0:SW, :])
                else:
                    nc.gpsimd.collective_compute(
                        "ReduceScatter",
                        mybir.AluOpType.add,
                        replica_groups=[[0, 1, 2, 3], [4, 5, 6, 7]],
                        ins=[rs_in_all.opt()],
                        outs=[y.opt()],
                    )


def build_nc(debug=False, single_core=False):
    nc = bacc.Bacc(
        "TRN2",
        target_bir_lowering=False,
        debug=debug,
        enable_asserts=True,
        num_devices=1 if single_core else NCORES,
        num_swdge_queues=4,
    )
    f32 = mybir.dt.float32
    bf16 = mybir.dt.bfloat16
    aps = {
        "xT": nc.dram_tensor("xT", [H, S], bf16, kind="ExternalInput").ap(),
        "wqkvT": nc.dram_tensor("wqkvT", [H, 768], bf16, kind="ExternalInput").ap(),
        "woTh": nc.dram_tensor("woTh", [512, H], bf16, kind="ExternalInput").ap(),
        "cos_t": nc.dram_tensor("cos_t", [P, S], bf16, kind="ExternalInput").ap(),
        "sin_t": nc.dram_tensor("sin_t", [P, S], bf16, kind="ExternalInput").ap(),
        "rotT": nc.dram_tensor("rotT", [P, P], bf16, kind="ExternalInput").ap(),
        "tri01": nc.dram_tensor("tri01", [P, P], bf16, kind="ExternalInput").ap(),
        "ones_t": nc.dram_tensor("ones_t", [P, P], bf16, kind="ExternalInput").ap(),
        "padb": nc.dram_tensor("padb", [P, HIDC], f32, kind="ExternalInput").ap(),
        "y": nc.dram_tensor("y", [SW, H], bf16, kind="ExternalOutput").ap(),
    }
    if single_core:
        aps["_single_core"] = True
    with tile.TileContext(nc) as tc:
        _emit(tc, aps)
    nc.compile()
    return nc


def _to_bf16(a):
    """Fast f32 -> bf16 cast (round-to-nearest-even) via bit manipulation."""
    u = np.ascontiguousarray(a, dtype=np.float32).view(np.uint32)
    r = ((u >> 16) & 1) + np.uint32(0x7FFF)
    return ((u + r) >> 16).astype(np.uint16).view(BF16)


_CONSTS = {}


def _const_tables():
    if _CONSTS:
        return _CONSTS
    pos = np.arange(S, dtype=np.float32)
    inv = 1.0 / THETA ** (np.arange(0, HD, 2, dtype=np.float32) / HD)  # [64]
    ang = inv[:, None] * pos[None, :]                 # [64, S]
    _CONSTS["cos_t"] = np.concatenate(
        [np.cos(ang), np.cos(ang)], axis=0).astype(BF16)
    _CONSTS["sin_t"] = np.concatenate(
        [np.sin(ang), np.sin(ang)], axis=0).astype(BF16)
    A = np.zeros((P, P), dtype=np.float32)
    i = np.arange(64)
    A[i, i + 64] = -1.0
    A[i + 64, i] = 1.0
    _CONSTS["rotT"] = np.ascontiguousarray(A.T).astype(BF16)
    # 0/1 keep-mask for the causal diagonal block: keep q >= k
    _CONSTS["tri01"] = np.where(
        np.arange(P)[None, :] >= np.arange(P)[:, None], 1.0, 0.0
    ).astype(BF16)
    _CONSTS["ones_t"] = np.ones((P, P), dtype=BF16)
    return _CONSTS


def host_inputs(hidden_states, attention_mask, wq, wk, wv, wo):
    """Build the per-core input maps (host-side sharding + constant tables)."""
    hs = np.asarray(hidden_states, dtype=np.float32)
    am = np.asarray(attention_mask)
    wq = np.asarray(wq, dtype=np.float32)
    wk = np.asarray(wk, dtype=np.float32)
    wv = np.asarray(wv, dtype=np.float32)
    wo = np.asarray(wo, dtype=np.float32)
    C = _const_tables()

    # per-batch: pre-transposed bf16 activations + pad bias (shared by 4 cores)
    xT_b, padb_b = [], []
    for b in range(B):
        xT_b.append(np.ascontiguousarray(_to_bf16(hs[b]).T))
        padb = np.where(
            am[b].astype(bool), 0.0, -1e30
        ).astype(np.float32).reshape(HIDC, P).T          # [P, HIDC]
        padb_b.append(np.ascontiguousarray(padb))

    # per-group: qkv + wo-rows weight slices (shared by both batches)
    wqkvT_g, woTh_g = [], []
    for g in range(NKV):
        wqT = wq[4 * g * HD:(4 * g + 4) * HD, :].T       # [H, 512]
        wkT = wk[g * HD:(g + 1) * HD, :].T               # [H, 128]
        wvT = wv[g * HD:(g + 1) * HD, :].T               # [H, 128]
        wqkvT_g.append(np.ascontiguousarray(
            np.concatenate([wqT, wkT, wvT], axis=1)).astype(BF16))
        woTh_g.append(
            np.ascontiguousarray(wo[:, 4 * g * HD:(4 * g + 4) * HD].T).astype(BF16))

    in_maps = []
    for core in range(NCORES):
        b, g = divmod(core, 4)
        in_maps.append(
            {
                "xT": xT_b[b],
                "wqkvT": wqkvT_g[g],
                "woTh": woTh_g[g],
                "cos_t": C["cos_t"],
                "sin_t": C["sin_t"],
                "rotT": C["rotT"],
                "tri01": C["tri01"],
                "ones_t": C["ones_t"],
                "padb": padb_b[b],
            }
        )
    return in_maps


def assemble(results):
    """Gather per-core outputs into the full [B, S, H] array.

    Core (b, g) owns tokens {512*w + 128*g + i} for w in 0..3: its y row
    block w holds the ReduceScattered (summed) output for those tokens.
    """
    out = np.empty((B, S, H), dtype=np.float32)
    for core in range(NCORES):
        b, g = divmod(core, 4)
        yc = np.asarray(results[core]["y"], dtype=np.float32)
        for w in range(4):
            r0 = 512 * w + 128 * g
            out[b, r0:r0 + P, :] = yc[w * P:(w + 1) * P, :]
    return out


_NC_CACHE = {}


def kernel(hidden_states, attention_mask, wq, wk, wv, wo, **run_kwargs):
    in_maps = host_inputs(hidden_states, attention_mask, wq, wk, wv, wo)
    if "nc" not in _NC_CACHE:
        _NC_CACHE["nc"] = build_nc(debug=False)
    nc = _NC_CACHE["nc"]
    res = run_bass_kernel_spmd(nc, in_maps, core_ids=list(range(NCORES)), **run_kwargs)
    out = assemble(res.results)
    kernel.last_results = res
    return out

